# revision 59
# baseline (speedup 1.0000x reference)
"""GAT (2-layer) on 8 Trainium2 NeuronCores — collective-minimal v2.

Strategy
--------
- Layer-1 node GEMM is REPLICATED: every core computes the full projected
  table t1_full[50176, 384] itself (rows: [h1 d-major 256 | a_src 8 | pad]),
  so no AllGather is needed before edge phase 1.  DMA-bound at ~112us,
  which beats the 364us modeled AllGather it replaces.
- Edges are routed to the core owning their destination (6250 nodes/core).
  Per dst block (128 nodes): two gather calls (src < s / src >= s so int16
  indices reach all 50176 rows), one-hot matrices via is_equal, transposed
  one-hots for the a_dst expansion, exp(leakyrelu(logits)) on Act written
  straight into the matmul rhs, messages = h1 * alpha with the d-major
  layout (DVE 2x mode), and a one-matmul-per-chunk scatter accumulation
  with the softmax denominators riding along as extra columns.
- h1 features are stored d-major ([d=32 x h=8] interleave) so the per-edge
  alpha broadcast lands on a middle axis and the multiply qualifies for the
  DVE 2x performance mode.
- Layer-2 table is collected PACKED (66 cols = 132B rows, 6.6MB vs 25.6MB)
  via a 5-chunk AllGather pipelined against layer-1 block production (small
  last chunk to shorten the tail), then restrided on-device into gatherable
  256B rows.  Collectives trigger 2 blocks after their input flush so the
  Pool SEQ never stalls gather issues; restrides are scheduled where their
  dependencies are already met so the in-order SP queue never blocks.
- The edge phase is software-pipelined 3 deep (one-hot prep at b+3, logit
  stage at b+1, message/scatter/finish at b) so the long cross-engine
  dependency chain overlaps across blocks despite in-order engine queues.
  All DMAs issue on the SP queue (an Activation-queue DMA issue blocks Act
  compute behind it).
"""

import math
import os

import numpy as np

import concourse.bass as bass
import concourse.mybir as mybir
import concourse.tile as tile
from concourse import bacc
from concourse.bass_utils import run_bass_kernel_spmd
from concourse.masks import make_identity

# problem constants (from the reference)
N = 50000
E = 500000
EMB = 128
HIDDEN = 256
HEADS = 8
OUT1 = 32
REPR = 64
NEG_SLOPE = 0.2

NC = 8
P = 128
NSH = N // NC                    # 6250 nodes per core
NBLK = (NSH + P - 1) // P        # 49 dst blocks per core
LASTB = NSH - (NBLK - 1) * P     # 106 nodes in last block
NROWS = 392 * P                  # 50176 padded table rows (identity node map)
HI_BASE = NROWS - 32768          # 17408: base row for the hi gather half

T1_ELEM = 384                    # fp16 row: h1(256 d-major) | a_src(8) | pad
T1_USED = 264
T2_ELEM = 128                    # fp16 row in t2_wide (256B gather stride)
T2_USED = 66                     # h2(64) | a2_src | a2_dst
MC1 = HIDDEN + HEADS             # 264 scatter cols layer 1
MC2 = REPR + 1                   # 65 scatter cols layer 2

F16 = mybir.dt.float16
F32 = mybir.dt.float32
I16 = mybir.dt.int16
AF = mybir.ActivationFunctionType
ALU = mybir.AluOpType

RING = 896                       # max idxs per gather call (1024-desc ring,
                                 # with the margin the ring needs in flight)
# t2 collective chunk boundaries (blocks; must be multiples of the 4-block
# staging flush so the shard rows are in DRAM before the collective reads them)
T2_CHUNKS = [(0, 12), (12, 24), (24, 36), (36, 44), (44, 48), (48, 49)]


def _ceil(a, b):
    return (a + b - 1) // b


def _prep_edges(edge_index):
    """Partition + sort edges; emit per-core idx/dl arrays + chunk metadata.

    Chunk metadata (kch_lo/kch_hi per block, call grouping) is shared across
    cores (max over cores) so the SPMD program is identical.
    """
    ei = np.asarray(edge_index)
    src = np.concatenate([ei[0], np.arange(N, dtype=np.int64)]).astype(np.int64)
    dst = np.concatenate([ei[1], np.arange(N, dtype=np.int64)]).astype(np.int64)
    core = dst // NSH
    dl = dst % NSH
    blk = dl // P
    dl128 = (dl % P).astype(np.float16)

    # counts per (core, block, half) for a candidate split s
    order0 = np.lexsort((src, blk, core))
    sc, bc, srt, dlc = core[order0], blk[order0], src[order0], dl128[order0]

    def counts_for(s):
        hi = (srt >= s).astype(np.int64)
        key = (sc * NBLK + bc) * 2 + hi
        return np.bincount(key, minlength=NC * NBLK * 2).reshape(NC, NBLK, 2)

    best = None
    for s in range(HI_BASE + 256, 32769, 256):
        c = counts_for(s)
        kl = _ceil(c[:, :, 0].max(axis=0), P)
        kh = _ceil(c[:, :, 1].max(axis=0), P)
        if kl.max() > 7 or kh.max() > 7:
            continue              # stay under the 1024-desc ring with margin
        tot = int(kl.sum() + kh.sum())
        if best is None or tot < best[0]:
            best = (tot, s, kl, kh)
    assert best is not None
    _, split_s, kch_lo, kch_hi = best
    kch_lo = kch_lo.astype(int)
    kch_hi = kch_hi.astype(int)

    # gather call grouping: pairs of blocks per half, <= RING idxs per call
    calls = []                    # flat list of (half, [blocks])
    pair_calls = []               # per pair: list of indices into `calls`
    for q in range(_ceil(NBLK, 2)):
        bs = [b for b in (2 * q, 2 * q + 1) if b < NBLK]
        mine = []
        for half, kch in ((0, kch_lo), (1, kch_hi)):
            tot = sum(int(kch[b]) for b in bs)
            if tot * P <= RING:
                mine.append(len(calls))
                calls.append((half, bs))
            else:
                for b in bs:
                    assert kch[b] * P <= RING
                    mine.append(len(calls))
                    calls.append((half, [b]))
        pair_calls.append(mine)

    cpb = kch_lo + kch_hi                       # chunks per block
    nchunk = int(cpb.sum())
    chunk_col = np.zeros(NBLK + 1, int)
    np.cumsum(cpb, out=chunk_col[1:])           # dl_T column of block b
    call_cols = []                              # idx col offset per call
    off = 0
    for half, bs in calls:
        call_cols.append(off)
        off += sum(int((kch_lo if half == 0 else kch_hi)[b]) for b in bs) * 8
    idx_cols = off

    idx_w = np.zeros((NC, 128, idx_cols), np.int16)
    dl_T = np.full((NC, 128, nchunk), 200.0, np.float16)
    for c in range(NC):
        m = core == c
        s_, d_, b_ = src[m], dl128[m], blk[m]
        hi_ = (s_ >= split_s)
        o = np.lexsort((s_, hi_.astype(np.int64), b_))
        s_, d_, b_, hi_ = s_[o], d_[o], b_[o], hi_[o]
        key = b_ * 2 + hi_
        cnt = np.bincount(key, minlength=NBLK * 2)
        starts = np.zeros(NBLK * 2 + 1, np.int64)
        np.cumsum(cnt, out=starts[1:])
        # per (block, half): idx values and dl columns
        for ci, (half, bs) in enumerate(calls):
            col0 = call_cols[ci]
            w = []
            for b in bs:
                kch = int((kch_lo if half == 0 else kch_hi)[b])
                g = b * 2 + half
                n = int(cnt[g])
                sl = slice(starts[g], starts[g] + n)
                iv = np.zeros(kch * P, np.int16)
                iv[:n] = (s_[sl] - (HI_BASE if half else 0)).astype(np.int16)
                w.append(iv)
                dv = np.full(kch * P, 200.0, np.float16)
                dv[:n] = d_[sl]
                cc = chunk_col[b] + (0 if half == 0 else int(kch_lo[b]))
                dl_T[c, :, cc:cc + kch] = dv.reshape(kch, P).T
            wv = np.concatenate(w)
            ww = wv.reshape(len(wv) // 16, 16).T       # wrap [i%16, i//16]
            idx_w[c, :, col0:col0 + ww.shape[1]] = np.tile(ww, (8, 1))
    meta = dict(split_s=split_s, kch_lo=kch_lo, kch_hi=kch_hi, calls=calls,
                pair_calls=pair_calls, call_cols=call_cols,
                chunk_col=chunk_col, idx_cols=idx_cols, nchunk=nchunk)
    return meta, idx_w, dl_T


def _build(meta):
    kch_lo, kch_hi = meta["kch_lo"], meta["kch_hi"]
    calls, call_cols = meta["calls"], meta["call_cols"]
    pair_calls = meta["pair_calls"]
    chunk_col = meta["chunk_col"]
    nchunk, idx_cols = meta["nchunk"], meta["idx_cols"]
    cpb = kch_lo + kch_hi
    CPBMAX = int(cpb.max())
    GMAX = max(sum(int((kch_lo if h == 0 else kch_hi)[b]) for b in bs)
               for h, bs in calls)

    nc = bacc.Bacc(None, target_bir_lowering=False)

    # ---- inputs (per core) ----
    embT_in = nc.dram_tensor("embT", [EMB, NROWS], F16, kind="ExternalInput")
    embm_in = nc.dram_tensor("embTm", [EMB, NBLK * P], F16, kind="ExternalInput")
    w1_in = nc.dram_tensor("w1ext", [EMB, 272], F16, kind="ExternalInput")
    w2_in = nc.dram_tensor("w2ext", [HIDDEN, T2_USED], F16, kind="ExternalInput")
    crow_in = nc.dram_tensor("crow", [P, T2_USED], F16, kind="ExternalInput")
    idx_in = nc.dram_tensor("idxw", [128, idx_cols], I16, kind="ExternalInput")
    dl_in = nc.dram_tensor("dlT", [128, nchunk], F16, kind="ExternalInput")
    iota_in = nc.dram_tensor("iota", [P, P], F16, kind="ExternalInput")
    out_t = nc.dram_tensor("out", [NSH, REPR], F32, kind="ExternalOutput")
    DBG = bool(os.environ.get("KDBG"))
    if DBG:
        dbg_t1 = nc.dram_tensor("dbg_t1", [256, T1_ELEM], F16, kind="ExternalOutput")
        dbg_adst1 = nc.dram_tensor("dbg_adst1", [P, NBLK * HEADS], F16, kind="ExternalOutput")
        dbg_pacc = nc.dram_tensor("dbg_pacc", [P, MC1], F32, kind="ExternalOutput")
        dbg_x16 = nc.dram_tensor("dbg_x16", [P, HIDDEN], F16, kind="ExternalOutput")
        dbg_t2s = nc.dram_tensor("dbg_t2s", [256, T2_USED], F16, kind="ExternalOutput")
        dbg_t2w = nc.dram_tensor("dbg_t2w", [256, T2_ELEM], F16, kind="ExternalOutput")

    # ---- internal DRAM ----
    t1_full = nc.dram_tensor("t1_full", [NROWS, T1_ELEM], F16, kind="Internal")
    t2_shard = nc.dram_tensor("t2_shard", [NSH, T2_USED], F16, kind="Internal")
    t2_pack = nc.dram_tensor("t2_pack", [N, T2_USED], F16, kind="Internal",
                             addr_space="Shared")
    t2_wide = nc.dram_tensor("t2_wide", [NROWS, T2_ELEM], F16, kind="Internal")

    with tile.TileContext(nc) as tc:
        with (
            tc.tile_pool(name="const", bufs=1) as cst,
            tc.tile_pool(name="sb", bufs=3) as sb,
            tc.tile_pool(name="gp", bufs=5) as gp,
            tc.tile_pool(name="rp", bufs=4) as rp,
            tc.tile_pool(name="sb2", bufs=3) as sb2,
            tc.tile_pool(name="oh", bufs=7) as ohp,
            tc.tile_pool(name="ohT", bufs=7) as ohtp,
            tc.tile_pool(name="psA", bufs=3, space="PSUM") as psA,
            tc.tile_pool(name="psB", bufs=2, space="PSUM") as psB,
            tc.tile_pool(name="psC", bufs=1, space="PSUM") as psC,
            tc.tile_pool(name="psD", bufs=2, space="PSUM") as psD,
        ):
            # ---- constants ----
            iota = cst.tile([P, P], F16)
            nc.sync.dma_start(out=iota[:], in_=iota_in[:])
            ident = cst.tile([P, P], F16)
            make_identity(nc, ident[:])
            w1 = cst.tile([EMB, 272], F16)
            nc.sync.dma_start(out=w1[:], in_=w1_in[:])
            w2 = cst.tile([P, 2, T2_USED], F16)
            nc.sync.dma_start(out=w2[:, 0, :], in_=w2_in[0:P, :])
            nc.sync.dma_start(out=w2[:, 1, :], in_=w2_in[P:HIDDEN, :])
            crow = cst.tile([P, T2_USED], F16)
            nc.sync.dma_start(out=crow[:], in_=crow_in[:])
            it_all = cst.tile([128, idx_cols], I16)
            nc.sync.dma_start(out=it_all[:], in_=idx_in[:])
            dl_all = cst.tile([128, nchunk], F16)
            nc.sync.dma_start(out=dl_all[:], in_=dl_in[:])
            adst1 = cst.tile([P, NBLK * HEADS], F16)
            adst2 = cst.tile([P, NBLK], F16)

            # ---- a_dst mini-GEMM over my own dst shard ----
            for g in range(7):
                em = sb.tile([EMB, 7, P], F16, tag="embm")
                nc.sync.dma_start(
                    out=em[:].rearrange("p a n -> p (a n)"),
                    in_=embm_in[:, g * 7 * P:(g + 1) * 7 * P])
                psx = psC.tile([P, 7, HEADS], F32, tag="pse")
                for j in range(7):
                    nc.tensor.matmul(out=psx[:, j, :], lhsT=em[:, j, :],
                                     rhs=w1[:, 264:272], start=True, stop=True)
                nc.vector.tensor_copy(
                    out=adst1[:, g * 56:(g + 1) * 56],
                    in_=psx[:].rearrange("p a h -> p (a h)"))

            # ---- phase A: replicated full-table GEMM1 (8 blocks/group) ----
            etq = {}

            def load_embT(g):
                et = sb.tile([EMB, 8, P], F16, tag="embT")
                nc.sync.dma_start(
                    out=et[:].rearrange("p a n -> p (a n)"),
                    in_=embT_in[:, g * 8 * P:(g + 1) * 8 * P])
                etq[g] = et

            load_embT(0)
            load_embT(1)
            for g in range(49):
                if g + 2 < 49:
                    load_embT(g + 2)
                et = etq.pop(g)
                t1s = sb.tile([P, 8, T1_USED], F16, tag="t1s")
                for j in range(8):
                    ph = psA.tile([P, T1_USED], F32, tag="acc")
                    nc.tensor.matmul(out=ph[:], lhsT=et[:, j, :],
                                     rhs=w1[:, 0:T1_USED], start=True, stop=True)
                    eng = (nc.vector, nc.scalar, nc.vector)[j % 3]
                    if eng is nc.scalar:
                        eng.copy(out=t1s[:, j, :], in_=ph[:])
                    else:
                        eng.tensor_copy(out=t1s[:, j, :], in_=ph[:])
                nc.sync.dma_start(
                    out=t1_full[g * 8 * P:(g + 1) * 8 * P, 0:T1_USED]
                        .rearrange("(a p) e -> p a e", p=P),
                    in_=t1s[:])

            if DBG:
                nc.sync.dma_start(out=dbg_t1[:], in_=t1_full[0:256, :])
                nc.sync.dma_start(out=dbg_adst1[:], in_=adst1[:])

            # ---- edge phase helper ----
            def edge_layer(t_full, elem, hid, heads, dmajor, adst_t, mcols,
                           out_cb):
                PREQ = 1              # pair prefetch distance
                gq = {}

                def issue_gathers(q):
                    gs = {}
                    for ci in pair_calls[q]:
                        half, bs = calls[ci]
                        kch = kch_lo if half == 0 else kch_hi
                        ch = sum(int(kch[b]) for b in bs)
                        gt = gp.tile([P, GMAX * T1_ELEM], F16, tag=f"g{half}")
                        col0 = call_cols[ci]
                        r0 = HI_BASE if half else 0
                        nc.gpsimd.dma_gather(
                            out_ap=gt[:, 0:ch * elem]
                                .rearrange("p (t e) -> p t e", e=elem),
                            in_ap=t_full[r0:NROWS, :],
                            idxs_ap=it_all[:, col0:col0 + ch * 8],
                            num_idxs=ch * P, num_idxs_reg=ch * P,
                            elem_size=elem)
                        off = 0
                        for b in bs:
                            gs[(b, half)] = (gt, off)
                            off += int(kch[b])
                    gq[q] = gs

                npairs = len(pair_calls)
                nextq = [0]

                def ensure_pairs(upto):
                    while nextq[0] <= min(upto, npairs - 1):
                        issue_gathers(nextq[0])
                        nextq[0] += 1

                def gv(b, half, c0, c1, e0, e1):
                    gt, off = gq[b // 2][(b, half)]
                    v = gt[:, 0:(off + c1) * elem].rearrange(
                        "p (t e) -> p t e", e=elem)
                    return v[:, off + c0:off + c1, e0:e1]

                state = {}

                def stage0(b):
                    """one-hots + transposed one-hots (needs only dl/iota)."""
                    t = int(cpb[b])
                    cc = int(chunk_col[b])
                    oh = ohp.tile([P, CPBMAX, P], F16, tag="oh")
                    nc.vector.tensor_tensor(
                        out=oh[:, 0:t, :],
                        in0=dl_all[:, cc:cc + t, None].to_broadcast([P, t, P]),
                        in1=iota[:, None, :].to_broadcast([P, t, P]),
                        op=ALU.is_equal)
                    ohT = ohtp.tile([P, CPBMAX, P], F16, tag="ohT")
                    for g0 in range(0, t, 8):
                        g1 = min(g0 + 8, t)
                        pst = psB.tile([P, 8, P], F16, tag="ohT")
                        for k in range(g0, g1):
                            nc.tensor.transpose(out=pst[:, k - g0, :],
                                                in_=oh[:, k, :],
                                                identity=ident[:])
                        eng = nc.scalar
                        if eng is nc.scalar:
                            eng.copy(
                                out=ohT[:, g0:g1, :].rearrange("p t n -> p (t n)"),
                                in_=pst[:, 0:g1 - g0, :]
                                    .rearrange("p t n -> p (t n)"))
                        else:
                            eng.tensor_copy(
                                out=ohT[:, g0:g1, :].rearrange("p t n -> p (t n)"),
                                in_=pst[:, 0:g1 - g0, :]
                                    .rearrange("p t n -> p (t n)"))
                    state[b] = dict(oh=oh, ohT=ohT)

                def stage1(b):
                    """logits -> exp weights (needs gathers + ohT)."""
                    st = state[b]
                    klo, khi = int(kch_lo[b]), int(kch_hi[b])
                    t = klo + khi
                    ohT = st["ohT"]
                    pse = psC.tile([P, CPBMAX * heads], F32, tag="pse")
                    for k in range(t):
                        half, ko = (0, k) if k < klo else (1, k - klo)
                        nc.tensor.matmul(
                            out=pse[:, k * heads:(k + 1) * heads],
                            lhsT=ohT[:, k, :],
                            rhs=adst_t[:, b * heads:(b + 1) * heads],
                            start=True, stop=False)
                        nc.tensor.matmul(
                            out=pse[:, k * heads:(k + 1) * heads],
                            lhsT=ident[:],
                            rhs=gv(b, half, ko, ko + 1, hid, hid + heads)[:, 0, :],
                            start=False, stop=True)
                    lk = sb2.tile([P, CPBMAX * heads], F32, tag=f"lk{hid}")
                    nc.scalar.activation(
                        out=lk[:, 0:t * heads],
                        in_=pse[:, 0:t * heads],
                        func=AF.Prelu, alpha=NEG_SLOPE)
                    rhs = rp.tile([P, CPBMAX, mcols], F16, tag="rhs")
                    nc.scalar.activation(
                        out=rhs[:, 0:t, hid:hid + heads],
                        in_=lk[:, 0:t * heads]
                            .rearrange("p (t h) -> p t h", h=heads),
                        func=AF.Exp)
                    st["rhs"] = rhs

                def stage2(b):
                    """messages + scatter + finish."""
                    st = state.pop(b)
                    klo, khi = int(kch_lo[b]), int(kch_hi[b])
                    t = klo + khi
                    oh, rhs = st["oh"], st["rhs"]
                    for half, k0, k1 in ((0, 0, klo), (1, klo, t)):
                        kk = k1 - k0
                        eng = nc.vector
                        if dmajor:
                            eng.tensor_tensor(
                                out=rhs[:, k0:k1, 0:hid]
                                    .rearrange("p t (d h) -> p t d h", h=heads),
                                in0=gv(b, half, 0, kk, 0, hid)
                                    .rearrange("p t (d h) -> p t d h", h=heads),
                                in1=rhs[:, k0:k1, None, hid:hid + heads]
                                    .to_broadcast([P, kk, hid // heads, heads]),
                                op=ALU.mult)
                        else:
                            eng.tensor_tensor(
                                out=rhs[:, k0:k1, 0:hid],
                                in0=gv(b, half, 0, kk, 0, hid),
                                in1=rhs[:, k0:k1, hid:hid + 1]
                                    .to_broadcast([P, kk, hid]),
                                op=ALU.mult)
                    pacc = psA.tile([P, mcols], F32, tag="acc")
                    for k in range(t):
                        nc.tensor.matmul(out=pacc[:], lhsT=oh[:, k, :],
                                         rhs=rhs[:, k, :], start=(k == 0),
                                         stop=(k == t - 1))
                    if b % 2 == 1:
                        gq.pop(b // 2)
                    out_cb(b, pacc)

                ensure_pairs(1)
                for b0w in range(5):
                    stage0(b0w)
                stage1(0)
                for b in range(NBLK):
                    ensure_pairs((b + 5) // 2)
                    if b + 5 < NBLK:
                        stage0(b + 5)
                    if b + 1 < NBLK:
                        stage1(b + 1)
                    stage2(b)

            # ---- layer 1 finisher ----
            def restride(ci):
                b0c, b1c = T2_CHUNKS[ci]
                r0, r1 = b0c * P, min(b1c * P, NSH)
                for c in range(NC):
                    nc.sync.dma_start(
                        out=t2_wide[c * NSH + r0:c * NSH + r1, 0:T2_USED],
                        in_=t2_pack[NC * r0 + c * (r1 - r0):
                                    NC * r0 + (c + 1) * (r1 - r0), :])

            t2s = {"tile": None, "b0": 0}

            def flush_t2(bend):
                tl, b0 = t2s["tile"], t2s["b0"]
                if tl is None:
                    return
                rows = min(bend * P, NSH) - b0 * P
                nc.sync.dma_start(
                    out=t2_shard[b0 * P:b0 * P + rows, :]
                        .rearrange("(a p) e -> p a e", p=P)
                    if rows % P == 0 else
                    t2_shard[b0 * P:b0 * P + rows, :],
                    in_=tl[:, 0:rows // P, :] if rows % P == 0
                    else tl[:rows, bend - 1 - b0, :])
                t2s["tile"] = None

            def finish1(b, pacc):
                if DBG and b == 0:
                    dpc = sb.tile([P, MC1], F32, tag="dpc")
                    nc.vector.tensor_copy(out=dpc[:], in_=pacc[:])
                    nc.sync.dma_start(out=dbg_pacc[:], in_=dpc[:])
                se = sb.tile([P, HEADS], F32, tag="se")
                nc.scalar.activation(out=se[:], in_=pacc[:, HIDDEN:MC1],
                                     func=AF.Copy, bias=1e-16)
                rec = sb.tile([P, HEADS], F32, tag="rec")
                nc.vector.reciprocal(out=rec[:], in_=se[:])
                v = sb.tile([P, HIDDEN], F32, tag="v")
                nc.vector.tensor_tensor(
                    out=v[:].rearrange("p (d h) -> p d h", h=HEADS),
                    in0=pacc[:, 0:HIDDEN].rearrange("p (d h) -> p d h", h=HEADS),
                    in1=rec[:, None, :].to_broadcast([P, OUT1, HEADS]),
                    op=ALU.mult)
                # elu(v) = relu(v) + exp(min(v,0)) - 1
                r = sb.tile([P, HIDDEN], F32, tag="relu")
                nc.scalar.activation(out=r[:], in_=v[:], func=AF.Relu)
                mn = sb.tile([P, HIDDEN], F32, tag="mn")
                nc.scalar.activation(out=mn[:], in_=v[:], func=AF.Relu,
                                     scale=-1.0)
                em = sb.tile([P, HIDDEN], F32, tag="em")
                nc.scalar.activation(out=em[:], in_=mn[:], func=AF.Exp,
                                     scale=-1.0)
                x = sb.tile([P, HIDDEN], F32, tag="x")
                nc.vector.tensor_tensor(out=x[:], in0=r[:], in1=em[:],
                                        op=ALU.add)
                x16 = sb.tile([P, HIDDEN], F16, tag="x16")
                nc.scalar.activation(out=x16[:], in_=x[:], func=AF.Copy,
                                     bias=-1.0)
                if DBG and b == 0:
                    nc.sync.dma_start(out=dbg_x16[:], in_=x16[:])
                xT = sb.tile([P, 2, P], F16, tag="xT")
                for k in range(2):
                    pst = psD.tile([P, P], F16, tag="misc")
                    nc.tensor.transpose(out=pst[:], in_=x16[:, k * P:(k + 1) * P],
                                        identity=ident[:])
                    nc.scalar.copy(out=xT[:, k, :], in_=pst[:])
                ph2 = psD.tile([P, T2_USED], F32, tag="misc")
                for k in range(2):
                    nc.tensor.matmul(out=ph2[:], lhsT=xT[:, k, :], rhs=w2[:, k, :],
                                     start=(k == 0), stop=(k == 1))
                if t2s["tile"] is None:
                    tl_new = sb.tile([P, 4, T2_USED], F16, tag="t2s")
                    t2s["tile"] = tl_new
                    t2s["b0"] = b
                tl = t2s["tile"]
                nc.scalar.copy(out=tl[:, b - t2s["b0"], :], in_=ph2[:])
                nc.scalar.activation(out=adst2[:, b:b + 1], in_=ph2[:, 65:66],
                                     func=AF.Copy)
                if b - t2s["b0"] == 3 or b == NBLK - 1:
                    flush_t2(b + 1)
                # t2 collective chunks ride block completion.  Collectives
                # must issue on the Pool queue; trigger each chunk 2 blocks
                # after its input flush so the Pool SEQ wait (which would
                # block later gather issues) is already satisfied.
                for ci, (b0c, b1c) in enumerate(T2_CHUNKS):
                    if b == (min(b1c + 1, NBLK - 1) if b1c <= 36 else b1c - 1):
                        r0, r1 = b0c * P, min(b1c * P, NSH)
                        nc.gpsimd.collective_compute(
                            "AllGather", ALU.bypass,
                            ins=[t2_shard[r0:r1, :]],
                            outs=[t2_pack[NC * r0:NC * r1, :]],
                            replica_groups=[list(range(NC))])
                # early chunks' restrides run mid-L1 once their collectives
                # are long done; the rest go after L1
                if b == 30:
                    restride(0)
                if b == 38:
                    restride(1)
                if b == 46:
                    restride(2)

            edge_layer(t1_full, T1_ELEM, HIDDEN, HEADS, True, adst1, MC1,
                       finish1)
            for ci in range(3, len(T2_CHUNKS)):
                restride(ci)
            if DBG:
                nc.sync.dma_start(out=dbg_t2s[:], in_=t2_shard[0:256, :])
                nc.sync.dma_start(out=dbg_t2w[:], in_=t2_wide[0:256, :])

            # ---- layer 2 finisher ----
            o4 = {"tile": None, "b0": 0}

            def finish2(b, pacc):
                se = sb.tile([P, 1], F32, tag="se2")
                nc.vector.tensor_scalar_add(out=se[:], in0=pacc[:, REPR:MC2],
                                            scalar1=1e-16)
                rec = sb.tile([P, 1], F32, tag="rec2")
                nc.vector.reciprocal(out=rec[:], in_=se[:])
                if o4["tile"] is None:
                    ot_new = sb.tile([P, 4, REPR], F32, tag="o4")
                    o4["tile"] = ot_new
                    o4["b0"] = b
                tl = o4["tile"]
                nc.scalar.activation(out=tl[:, b - o4["b0"], :],
                                     in_=pacc[:, 0:REPR], func=AF.Copy,
                                     scale=rec[:, 0:1])
                if b - o4["b0"] == 3 or b == NBLK - 1:
                    b0 = o4["b0"]
                    rows = min((b + 1) * P, NSH) - b0 * P
                    if rows % P == 0:
                        nc.sync.dma_start(
                            out=out_t[b0 * P:b0 * P + rows, :]
                                .rearrange("(a p) e -> p a e", p=P),
                            in_=tl[:, 0:rows // P, :])
                    else:
                        full = rows // P
                        if full:
                            nc.sync.dma_start(
                                out=out_t[b0 * P:b0 * P + full * P, :]
                                    .rearrange("(a p) e -> p a e", p=P),
                                in_=tl[:, 0:full, :])
                        nc.sync.dma_start(
                            out=out_t[b0 * P + full * P:b0 * P + rows, :],
                            in_=tl[:rows - full * P, full, :])
                    o4["tile"] = None

            edge_layer(t2_wide, T2_ELEM, REPR, 1, False, adst2, MC2,
                       finish2)

    nc.finalize()
    globals()["LAST_NC"] = nc
    return nc


def kernel(**inputs):
    node_emb = np.asarray(inputs["node_emb"], np.float32)
    W1 = np.asarray(inputs["W1"], np.float32)
    att1_src = np.asarray(inputs["att1_src"], np.float32)
    att1_dst = np.asarray(inputs["att1_dst"], np.float32)
    b1 = np.asarray(inputs["b1"], np.float32)
    W2 = np.asarray(inputs["W2"], np.float32)
    att2_src = np.asarray(inputs["att2_src"], np.float32)
    att2_dst = np.asarray(inputs["att2_dst"], np.float32)
    b2 = np.asarray(inputs["b2"], np.float32)
    edge_index = np.asarray(inputs["edge_index"])
    assert not np.any(b1) and not np.any(b2), "nonzero biases unsupported"

    meta, idx_w, dl_T = _prep_edges(edge_index)

    # d-major permutation: table col d*8+h <- W1 output col h*32+d
    dd, hh = np.meshgrid(np.arange(OUT1), np.arange(HEADS), indexing="ij")
    src_cols = (hh * OUT1 + dd).reshape(-1)          # length 256, j=d*8+h
    A1s = np.einsum("ehd,hd->eh", W1.reshape(EMB, HEADS, OUT1), att1_src)
    A1d = np.einsum("ehd,hd->eh", W1.reshape(EMB, HEADS, OUT1), att1_dst)
    w1ext = np.concatenate([W1[:, src_cols], A1s, A1d], axis=1).astype(np.float16)
    A2s = W2 @ att2_src[0]
    A2d = W2 @ att2_dst[0]
    w2e_rows = np.concatenate([W2, A2s[:, None], A2d[:, None]], axis=1)
    w2ext = w2e_rows[src_cols, :].astype(np.float16)   # rows follow x16 cols
    # x16 = elu(v)+1, so GEMM2 adds a constant row correction -sum(W2ext rows)
    # injected via matmul(lhsT=ident, rhs=crow).
    crow = np.tile((-w2ext.astype(np.float32).sum(axis=0))[None, :],
                   (P, 1)).astype(np.float16)

    iota = np.tile(np.arange(P, dtype=np.float16), (P, 1))

    nc = _build(meta)

    embT = np.zeros((EMB, NROWS), np.float16)
    embT[:, :N] = node_emb.T.astype(np.float16)
    embm = np.zeros((NC, EMB, NBLK * P), np.float16)
    for c in range(NC):
        embm[c, :, :NSH] = node_emb[c * NSH:(c + 1) * NSH].T.astype(np.float16)

    in_maps = []
    for c in range(NC):
        in_maps.append({
            "embT": embT,
            "embTm": embm[c],
            "w1ext": w1ext,
            "w2ext": w2ext,
            "crow": crow,
            "idxw": idx_w[c],
            "dlT": dl_T[c],
            "iota": iota,
        })

    res = run_bass_kernel_spmd(nc, in_maps, core_ids=list(range(NC)))
    globals()["LAST_RES"] = res
    globals()["LAST_META"] = meta
    out = np.concatenate([res.results[c]["out"] for c in range(NC)], axis=0)
    return np.ascontiguousarray(out.astype(np.float32))


if __name__ == "__main__":
    rng = np.random.default_rng(0)
    ins = {
        "node_emb": rng.standard_normal((N, EMB), dtype=np.float32) * 0.05,
        "W1": rng.standard_normal((EMB, HIDDEN), dtype=np.float32) * 0.07,
        "att1_src": rng.standard_normal((HEADS, OUT1), dtype=np.float32) * 0.2,
        "att1_dst": rng.standard_normal((HEADS, OUT1), dtype=np.float32) * 0.2,
        "b1": np.zeros(HIDDEN, np.float32),
        "W2": rng.standard_normal((HIDDEN, REPR), dtype=np.float32) * 0.07,
        "att2_src": rng.standard_normal((1, REPR), dtype=np.float32) * 0.2,
        "att2_dst": rng.standard_normal((1, REPR), dtype=np.float32) * 0.2,
        "b2": np.zeros(REPR, np.float32),
        "edge_index": rng.integers(0, N, (2, E)).astype(np.int32),
    }
    out = kernel(**ins)
    print("out", out.shape, out.dtype, np.abs(out).mean())


# revision 60
# speedup vs baseline: 1.0492x; 1.0492x over previous
"""GAT (2-layer) on 8 Trainium2 NeuronCores — collective-minimal v2.

Strategy
--------
- Layer-1 node GEMM is REPLICATED: every core computes the full projected
  table t1_full[50176, 384] itself (rows: [h1 d-major 256 | a_src 8 | pad]),
  so no AllGather is needed before edge phase 1.  DMA-bound at ~112us,
  which beats the 364us modeled AllGather it replaces.
- Edges are routed to the core owning their destination (6250 nodes/core).
  Per dst block (128 nodes): two gather calls (src < s / src >= s so int16
  indices reach all 50176 rows), one-hot matrices via is_equal, transposed
  one-hots for the a_dst expansion, exp(leakyrelu(logits)) on Act written
  straight into the matmul rhs, messages = h1 * alpha with the d-major
  layout (DVE 2x mode), and a one-matmul-per-chunk scatter accumulation
  with the softmax denominators riding along as extra columns.
- h1 features are stored d-major ([d=32 x h=8] interleave) so the per-edge
  alpha broadcast lands on a middle axis and the multiply qualifies for the
  DVE 2x performance mode.
- Layer-2 table is collected PACKED (66 cols = 132B rows, 6.6MB vs 25.6MB)
  via a 5-chunk AllGather pipelined against layer-1 block production (small
  last chunk to shorten the tail), then restrided on-device into gatherable
  256B rows.  Collectives trigger 2 blocks after their input flush so the
  Pool SEQ never stalls gather issues; restrides are scheduled where their
  dependencies are already met so the in-order SP queue never blocks.
- The edge phase is software-pipelined 3 deep (one-hot prep at b+3, logit
  stage at b+1, message/scatter/finish at b) so the long cross-engine
  dependency chain overlaps across blocks despite in-order engine queues.
  All DMAs issue on the SP queue (an Activation-queue DMA issue blocks Act
  compute behind it).
"""

import math
import os

import numpy as np

import concourse.bass as bass
import concourse.mybir as mybir
import concourse.tile as tile
from concourse import bacc
from concourse.bass_utils import run_bass_kernel_spmd
from concourse.masks import make_identity

# problem constants (from the reference)
N = 50000
E = 500000
EMB = 128
HIDDEN = 256
HEADS = 8
OUT1 = 32
REPR = 64
NEG_SLOPE = 0.2

NC = 8
P = 128
NSH = N // NC                    # 6250 nodes per core
NBLK = (NSH + P - 1) // P        # 49 dst blocks per core
LASTB = NSH - (NBLK - 1) * P     # 106 nodes in last block
NROWS = 392 * P                  # 50176 padded table rows (identity node map)
HI_BASE = NROWS - 32768          # 17408: base row for the hi gather half

T1_ELEM = 384                    # fp16 row: h1(256 d-major) | a_src(8) | pad
T1_USED = 264
T2_ELEM = 128                    # fp16 row in t2_wide (256B gather stride)
T2_USED = 66                     # h2(64) | a2_src | a2_dst
MC1 = HIDDEN + HEADS             # 264 scatter cols layer 1
MC2 = REPR + 1                   # 65 scatter cols layer 2

F16 = mybir.dt.float16
F32 = mybir.dt.float32
I16 = mybir.dt.int16
AF = mybir.ActivationFunctionType
ALU = mybir.AluOpType

RING = 896                       # max idxs per gather call (1024-desc ring,
                                 # with the margin the ring needs in flight)
# t2 collective chunk boundaries (blocks; must be multiples of the 4-block
# staging flush so the shard rows are in DRAM before the collective reads them)
T2_CHUNKS = [(0, 12), (12, 24), (24, 36), (36, 44), (44, 48), (48, 49)]


def _ceil(a, b):
    return (a + b - 1) // b


def _prep_edges(edge_index):
    """Partition + sort edges; emit per-core idx/dl arrays + chunk metadata.

    Chunk metadata (kch_lo/kch_hi per block, call grouping) is shared across
    cores (max over cores) so the SPMD program is identical.
    """
    ei = np.asarray(edge_index)
    src = np.concatenate([ei[0], np.arange(N, dtype=np.int64)]).astype(np.int64)
    dst = np.concatenate([ei[1], np.arange(N, dtype=np.int64)]).astype(np.int64)
    core = dst // NSH
    dl = dst % NSH
    blk = dl // P
    dl128 = (dl % P).astype(np.float16)

    # counts per (core, block, half) for a candidate split s
    order0 = np.lexsort((src, blk, core))
    sc, bc, srt, dlc = core[order0], blk[order0], src[order0], dl128[order0]

    def counts_for(s):
        hi = (srt >= s).astype(np.int64)
        key = (sc * NBLK + bc) * 2 + hi
        return np.bincount(key, minlength=NC * NBLK * 2).reshape(NC, NBLK, 2)

    best = None
    for s in range(HI_BASE + 256, 32769, 256):
        c = counts_for(s)
        kl = _ceil(c[:, :, 0].max(axis=0), P)
        kh = _ceil(c[:, :, 1].max(axis=0), P)
        if kl.max() > 7 or kh.max() > 7:
            continue              # stay under the 1024-desc ring with margin
        tot = int(kl.sum() + kh.sum())
        if best is None or tot < best[0]:
            best = (tot, s, kl, kh)
    assert best is not None
    _, split_s, kch_lo, kch_hi = best
    kch_lo = kch_lo.astype(int)
    kch_hi = kch_hi.astype(int)

    # gather call grouping: pairs of blocks per half, <= RING idxs per call
    calls = []                    # flat list of (half, [blocks])
    pair_calls = []               # per pair: list of indices into `calls`
    for q in range(_ceil(NBLK, 2)):
        bs = [b for b in (2 * q, 2 * q + 1) if b < NBLK]
        mine = []
        for half, kch in ((0, kch_lo), (1, kch_hi)):
            tot = sum(int(kch[b]) for b in bs)
            if tot * P <= RING:
                mine.append(len(calls))
                calls.append((half, bs))
            else:
                for b in bs:
                    assert kch[b] * P <= RING
                    mine.append(len(calls))
                    calls.append((half, [b]))
        pair_calls.append(mine)

    cpb = kch_lo + kch_hi                       # chunks per block
    nchunk = int(cpb.sum())
    chunk_col = np.zeros(NBLK + 1, int)
    np.cumsum(cpb, out=chunk_col[1:])           # dl_T column of block b
    call_cols = []                              # idx col offset per call
    off = 0
    for half, bs in calls:
        call_cols.append(off)
        off += sum(int((kch_lo if half == 0 else kch_hi)[b]) for b in bs) * 8
    idx_cols = off

    idx_w = np.zeros((NC, 128, idx_cols), np.int16)
    dl_T = np.full((NC, 128, nchunk), 200.0, np.float16)
    for c in range(NC):
        m = core == c
        s_, d_, b_ = src[m], dl128[m], blk[m]
        hi_ = (s_ >= split_s)
        o = np.lexsort((s_, hi_.astype(np.int64), b_))
        s_, d_, b_, hi_ = s_[o], d_[o], b_[o], hi_[o]
        key = b_ * 2 + hi_
        cnt = np.bincount(key, minlength=NBLK * 2)
        starts = np.zeros(NBLK * 2 + 1, np.int64)
        np.cumsum(cnt, out=starts[1:])
        # per (block, half): idx values and dl columns
        for ci, (half, bs) in enumerate(calls):
            col0 = call_cols[ci]
            w = []
            for b in bs:
                kch = int((kch_lo if half == 0 else kch_hi)[b])
                g = b * 2 + half
                n = int(cnt[g])
                sl = slice(starts[g], starts[g] + n)
                iv = np.zeros(kch * P, np.int16)
                iv[:n] = (s_[sl] - (HI_BASE if half else 0)).astype(np.int16)
                w.append(iv)
                dv = np.full(kch * P, 200.0, np.float16)
                dv[:n] = d_[sl]
                cc = chunk_col[b] + (0 if half == 0 else int(kch_lo[b]))
                dl_T[c, :, cc:cc + kch] = dv.reshape(kch, P).T
            wv = np.concatenate(w)
            ww = wv.reshape(len(wv) // 16, 16).T       # wrap [i%16, i//16]
            idx_w[c, :, col0:col0 + ww.shape[1]] = np.tile(ww, (8, 1))
    meta = dict(split_s=split_s, kch_lo=kch_lo, kch_hi=kch_hi, calls=calls,
                pair_calls=pair_calls, call_cols=call_cols,
                chunk_col=chunk_col, idx_cols=idx_cols, nchunk=nchunk)
    return meta, idx_w, dl_T


def _build(meta):
    kch_lo, kch_hi = meta["kch_lo"], meta["kch_hi"]
    calls, call_cols = meta["calls"], meta["call_cols"]
    pair_calls = meta["pair_calls"]
    chunk_col = meta["chunk_col"]
    nchunk, idx_cols = meta["nchunk"], meta["idx_cols"]
    cpb = kch_lo + kch_hi
    CPBMAX = int(cpb.max())
    GMAX = max(sum(int((kch_lo if h == 0 else kch_hi)[b]) for b in bs)
               for h, bs in calls)

    nc = bacc.Bacc(None, target_bir_lowering=False)

    # ---- inputs (per core) ----
    embT_in = nc.dram_tensor("embT", [EMB, NROWS], F16, kind="ExternalInput")
    embm_in = nc.dram_tensor("embTm", [EMB, NBLK * P], F16, kind="ExternalInput")
    w1_in = nc.dram_tensor("w1ext", [EMB, 272], F16, kind="ExternalInput")
    w2_in = nc.dram_tensor("w2ext", [HIDDEN, T2_USED], F16, kind="ExternalInput")
    crow_in = nc.dram_tensor("crow", [P, T2_USED], F16, kind="ExternalInput")
    idx_in = nc.dram_tensor("idxw", [128, idx_cols], I16, kind="ExternalInput")
    dl_in = nc.dram_tensor("dlT", [128, nchunk], F16, kind="ExternalInput")
    iota_in = nc.dram_tensor("iota", [P, P], F16, kind="ExternalInput")
    out_t = nc.dram_tensor("out", [NSH, REPR], F32, kind="ExternalOutput")
    DBG = bool(os.environ.get("KDBG"))
    if DBG:
        dbg_t1 = nc.dram_tensor("dbg_t1", [256, T1_ELEM], F16, kind="ExternalOutput")
        dbg_adst1 = nc.dram_tensor("dbg_adst1", [P, NBLK * HEADS], F16, kind="ExternalOutput")
        dbg_pacc = nc.dram_tensor("dbg_pacc", [P, MC1], F32, kind="ExternalOutput")
        dbg_x16 = nc.dram_tensor("dbg_x16", [P, HIDDEN], F16, kind="ExternalOutput")
        dbg_t2s = nc.dram_tensor("dbg_t2s", [256, T2_USED], F16, kind="ExternalOutput")
        dbg_t2w = nc.dram_tensor("dbg_t2w", [256, T2_ELEM], F16, kind="ExternalOutput")

    # ---- internal DRAM ----
    t1_full = nc.dram_tensor("t1_full", [NROWS, T1_ELEM], F16, kind="Internal")
    t2_shard = nc.dram_tensor("t2_shard", [NSH, T2_USED], F16, kind="Internal")
    t2_pack = nc.dram_tensor("t2_pack", [N, T2_USED], F16, kind="Internal",
                             addr_space="Shared")
    t2_wide = nc.dram_tensor("t2_wide", [NROWS, T2_ELEM], F16, kind="Internal")

    with tile.TileContext(nc) as tc:
        with (
            tc.tile_pool(name="const", bufs=1) as cst,
            tc.tile_pool(name="sb", bufs=3) as sb,
            tc.tile_pool(name="gp", bufs=5) as gp,
            tc.tile_pool(name="rp", bufs=4) as rp,
            tc.tile_pool(name="sb2", bufs=3) as sb2,
            tc.tile_pool(name="oh", bufs=5) as ohp,
            tc.tile_pool(name="ohT", bufs=5) as ohtp,
            tc.tile_pool(name="psA", bufs=3, space="PSUM") as psA,
            tc.tile_pool(name="psB", bufs=2, space="PSUM") as psB,
            tc.tile_pool(name="psC", bufs=1, space="PSUM") as psC,
            tc.tile_pool(name="psD", bufs=2, space="PSUM") as psD,
        ):
            # ---- constants ----
            iota = cst.tile([P, P], F16)
            nc.sync.dma_start(out=iota[:], in_=iota_in[:])
            ident = cst.tile([P, P], F16)
            make_identity(nc, ident[:])
            w1 = cst.tile([EMB, 272], F16)
            nc.sync.dma_start(out=w1[:], in_=w1_in[:])
            w2 = cst.tile([P, 2, T2_USED], F16)
            nc.sync.dma_start(out=w2[:, 0, :], in_=w2_in[0:P, :])
            nc.sync.dma_start(out=w2[:, 1, :], in_=w2_in[P:HIDDEN, :])
            crow = cst.tile([P, T2_USED], F16)
            nc.sync.dma_start(out=crow[:], in_=crow_in[:])
            it_all = cst.tile([128, idx_cols], I16)
            nc.sync.dma_start(out=it_all[:], in_=idx_in[:])
            dl_all = cst.tile([128, nchunk], F16)
            nc.sync.dma_start(out=dl_all[:], in_=dl_in[:])
            adst1 = cst.tile([P, NBLK * HEADS], F16)
            adst2 = cst.tile([P, NBLK], F16)

            # ---- a_dst mini-GEMM over my own dst shard ----
            for g in range(7):
                em = sb.tile([EMB, 7, P], F16, tag="embm")
                nc.sync.dma_start(
                    out=em[:].rearrange("p a n -> p (a n)"),
                    in_=embm_in[:, g * 7 * P:(g + 1) * 7 * P])
                psx = psC.tile([P, 7, HEADS], F32, tag="pse")
                for j in range(7):
                    nc.tensor.matmul(out=psx[:, j, :], lhsT=em[:, j, :],
                                     rhs=w1[:, 264:272], start=True, stop=True)
                nc.vector.tensor_copy(
                    out=adst1[:, g * 56:(g + 1) * 56],
                    in_=psx[:].rearrange("p a h -> p (a h)"))

            # ---- phase A: replicated full-table GEMM1 (8 blocks/group) ----
            etq = {}

            def load_embT(g):
                et = sb.tile([EMB, 8, P], F16, tag="embT")
                nc.sync.dma_start(
                    out=et[:].rearrange("p a n -> p (a n)"),
                    in_=embT_in[:, g * 8 * P:(g + 1) * 8 * P])
                etq[g] = et

            load_embT(0)
            load_embT(1)
            for g in range(49):
                if g + 2 < 49:
                    load_embT(g + 2)
                et = etq.pop(g)
                t1s = sb.tile([P, 8, T1_USED], F16, tag="t1s")
                for j in range(8):
                    ph = psA.tile([P, T1_USED], F32, tag="acc")
                    nc.tensor.matmul(out=ph[:], lhsT=et[:, j, :],
                                     rhs=w1[:, 0:T1_USED], start=True, stop=True)
                    eng = (nc.vector, nc.scalar, nc.vector)[j % 3]
                    if eng is nc.scalar:
                        eng.copy(out=t1s[:, j, :], in_=ph[:])
                    else:
                        eng.tensor_copy(out=t1s[:, j, :], in_=ph[:])
                nc.sync.dma_start(
                    out=t1_full[g * 8 * P:(g + 1) * 8 * P, 0:T1_USED]
                        .rearrange("(a p) e -> p a e", p=P),
                    in_=t1s[:])

            if DBG:
                nc.sync.dma_start(out=dbg_t1[:], in_=t1_full[0:256, :])
                nc.sync.dma_start(out=dbg_adst1[:], in_=adst1[:])

            # ---- edge phase helper ----
            def edge_layer(t_full, elem, hid, heads, dmajor, adst_t, mcols,
                           out_cb):
                PREQ = 1              # pair prefetch distance
                gq = {}

                def issue_gathers(q):
                    gs = {}
                    for ci in pair_calls[q]:
                        half, bs = calls[ci]
                        kch = kch_lo if half == 0 else kch_hi
                        ch = sum(int(kch[b]) for b in bs)
                        gt = gp.tile([P, GMAX * T1_ELEM], F16, tag=f"g{half}")
                        col0 = call_cols[ci]
                        r0 = HI_BASE if half else 0
                        nc.gpsimd.dma_gather(
                            out_ap=gt[:, 0:ch * elem]
                                .rearrange("p (t e) -> p t e", e=elem),
                            in_ap=t_full[r0:NROWS, :],
                            idxs_ap=it_all[:, col0:col0 + ch * 8],
                            num_idxs=ch * P, num_idxs_reg=ch * P,
                            elem_size=elem)
                        off = 0
                        for b in bs:
                            gs[(b, half)] = (gt, off)
                            off += int(kch[b])
                    gq[q] = gs

                npairs = len(pair_calls)
                nextq = [0]

                def ensure_pairs(upto):
                    while nextq[0] <= min(upto, npairs - 1):
                        issue_gathers(nextq[0])
                        nextq[0] += 1

                def gv(b, half, c0, c1, e0, e1):
                    gt, off = gq[b // 2][(b, half)]
                    v = gt[:, 0:(off + c1) * elem].rearrange(
                        "p (t e) -> p t e", e=elem)
                    return v[:, off + c0:off + c1, e0:e1]

                state = {}

                def stage0(b):
                    """one-hots + transposed one-hots (needs only dl/iota)."""
                    t = int(cpb[b])
                    cc = int(chunk_col[b])
                    oh = ohp.tile([P, CPBMAX, P], F16, tag="oh")
                    nc.vector.tensor_tensor(
                        out=oh[:, 0:t, :],
                        in0=dl_all[:, cc:cc + t, None].to_broadcast([P, t, P]),
                        in1=iota[:, None, :].to_broadcast([P, t, P]),
                        op=ALU.is_equal)
                    ohT = ohtp.tile([P, CPBMAX, P], F16, tag="ohT")
                    for g0 in range(0, t, 8):
                        g1 = min(g0 + 8, t)
                        pst = psB.tile([P, 8, P], F16, tag="ohT")
                        for k in range(g0, g1):
                            nc.tensor.transpose(out=pst[:, k - g0, :],
                                                in_=oh[:, k, :],
                                                identity=ident[:])
                        eng = nc.scalar
                        if eng is nc.scalar:
                            eng.copy(
                                out=ohT[:, g0:g1, :].rearrange("p t n -> p (t n)"),
                                in_=pst[:, 0:g1 - g0, :]
                                    .rearrange("p t n -> p (t n)"))
                        else:
                            eng.tensor_copy(
                                out=ohT[:, g0:g1, :].rearrange("p t n -> p (t n)"),
                                in_=pst[:, 0:g1 - g0, :]
                                    .rearrange("p t n -> p (t n)"))
                    state[b] = dict(oh=oh, ohT=ohT)

                def stage1(b):
                    """logits -> exp weights (needs gathers + ohT)."""
                    st = state[b]
                    klo, khi = int(kch_lo[b]), int(kch_hi[b])
                    t = klo + khi
                    ohT = st["ohT"]
                    pse = psC.tile([P, CPBMAX * heads], F32, tag="pse")
                    for k in range(t):
                        half, ko = (0, k) if k < klo else (1, k - klo)
                        nc.tensor.matmul(
                            out=pse[:, k * heads:(k + 1) * heads],
                            lhsT=ohT[:, k, :],
                            rhs=adst_t[:, b * heads:(b + 1) * heads],
                            start=True, stop=False)
                        nc.tensor.matmul(
                            out=pse[:, k * heads:(k + 1) * heads],
                            lhsT=ident[:],
                            rhs=gv(b, half, ko, ko + 1, hid, hid + heads)[:, 0, :],
                            start=False, stop=True)
                    lk = sb2.tile([P, CPBMAX * heads], F32, tag=f"lk{hid}")
                    nc.scalar.activation(
                        out=lk[:, 0:t * heads],
                        in_=pse[:, 0:t * heads],
                        func=AF.Prelu, alpha=NEG_SLOPE)
                    rhs = rp.tile([P, CPBMAX, mcols], F16, tag="rhs")
                    nc.scalar.activation(
                        out=rhs[:, 0:t, hid:hid + heads],
                        in_=lk[:, 0:t * heads]
                            .rearrange("p (t h) -> p t h", h=heads),
                        func=AF.Exp)
                    st["rhs"] = rhs

                def stage2(b):
                    """messages + scatter + finish."""
                    st = state.pop(b)
                    klo, khi = int(kch_lo[b]), int(kch_hi[b])
                    t = klo + khi
                    oh, rhs = st["oh"], st["rhs"]
                    for half, k0, k1 in ((0, 0, klo), (1, klo, t)):
                        kk = k1 - k0
                        eng = nc.vector
                        if dmajor:
                            eng.tensor_tensor(
                                out=rhs[:, k0:k1, 0:hid]
                                    .rearrange("p t (d h) -> p t d h", h=heads),
                                in0=gv(b, half, 0, kk, 0, hid)
                                    .rearrange("p t (d h) -> p t d h", h=heads),
                                in1=rhs[:, k0:k1, None, hid:hid + heads]
                                    .to_broadcast([P, kk, hid // heads, heads]),
                                op=ALU.mult)
                        else:
                            eng.tensor_tensor(
                                out=rhs[:, k0:k1, 0:hid],
                                in0=gv(b, half, 0, kk, 0, hid),
                                in1=rhs[:, k0:k1, hid:hid + 1]
                                    .to_broadcast([P, kk, hid]),
                                op=ALU.mult)
                    pacc = psA.tile([P, mcols], F32, tag="acc")
                    for k in range(t):
                        nc.tensor.matmul(out=pacc[:], lhsT=oh[:, k, :],
                                         rhs=rhs[:, k, :], start=(k == 0),
                                         stop=(k == t - 1))
                    if b % 2 == 1:
                        gq.pop(b // 2)
                    out_cb(b, pacc)

                ensure_pairs(1)
                for b0w in range(4):
                    stage0(b0w)
                stage1(0)
                for b in range(NBLK):
                    ensure_pairs((b + 5) // 2)
                    if b + 4 < NBLK:
                        stage0(b + 4)
                    if b + 1 < NBLK:
                        stage1(b + 1)
                    stage2(b)

            # ---- layer 1 finisher ----
            def restride(ci):
                b0c, b1c = T2_CHUNKS[ci]
                r0, r1 = b0c * P, min(b1c * P, NSH)
                for c in range(NC):
                    nc.sync.dma_start(
                        out=t2_wide[c * NSH + r0:c * NSH + r1, 0:T2_USED],
                        in_=t2_pack[NC * r0 + c * (r1 - r0):
                                    NC * r0 + (c + 1) * (r1 - r0), :])

            t2s = {"tile": None, "b0": 0}

            def flush_t2(bend):
                tl, b0 = t2s["tile"], t2s["b0"]
                if tl is None:
                    return
                rows = min(bend * P, NSH) - b0 * P
                nc.sync.dma_start(
                    out=t2_shard[b0 * P:b0 * P + rows, :]
                        .rearrange("(a p) e -> p a e", p=P)
                    if rows % P == 0 else
                    t2_shard[b0 * P:b0 * P + rows, :],
                    in_=tl[:, 0:rows // P, :] if rows % P == 0
                    else tl[:rows, bend - 1 - b0, :])
                t2s["tile"] = None

            def finish1(b, pacc):
                if DBG and b == 0:
                    dpc = sb.tile([P, MC1], F32, tag="dpc")
                    nc.vector.tensor_copy(out=dpc[:], in_=pacc[:])
                    nc.sync.dma_start(out=dbg_pacc[:], in_=dpc[:])
                se = sb.tile([P, HEADS], F32, tag="se")
                nc.scalar.activation(out=se[:], in_=pacc[:, HIDDEN:MC1],
                                     func=AF.Copy, bias=1e-16)
                rec = sb.tile([P, HEADS], F32, tag="rec")
                nc.vector.reciprocal(out=rec[:], in_=se[:])
                v = sb.tile([P, HIDDEN], F32, tag="v")
                nc.vector.tensor_tensor(
                    out=v[:].rearrange("p (d h) -> p d h", h=HEADS),
                    in0=pacc[:, 0:HIDDEN].rearrange("p (d h) -> p d h", h=HEADS),
                    in1=rec[:, None, :].to_broadcast([P, OUT1, HEADS]),
                    op=ALU.mult)
                # elu(v) = relu(v) + exp(min(v,0)) - 1
                r = sb.tile([P, HIDDEN], F32, tag="relu")
                nc.scalar.activation(out=r[:], in_=v[:], func=AF.Relu)
                mn = sb.tile([P, HIDDEN], F32, tag="mn")
                nc.scalar.activation(out=mn[:], in_=v[:], func=AF.Relu,
                                     scale=-1.0)
                em = sb.tile([P, HIDDEN], F32, tag="em")
                nc.scalar.activation(out=em[:], in_=mn[:], func=AF.Exp,
                                     scale=-1.0)
                x = sb.tile([P, HIDDEN], F32, tag="x")
                nc.vector.tensor_tensor(out=x[:], in0=r[:], in1=em[:],
                                        op=ALU.add)
                x16 = sb.tile([P, HIDDEN], F16, tag="x16")
                nc.scalar.activation(out=x16[:], in_=x[:], func=AF.Copy,
                                     bias=-1.0)
                if DBG and b == 0:
                    nc.sync.dma_start(out=dbg_x16[:], in_=x16[:])
                xT = sb.tile([P, 2, P], F16, tag="xT")
                for k in range(2):
                    pst = psD.tile([P, P], F16, tag="misc")
                    nc.tensor.transpose(out=pst[:], in_=x16[:, k * P:(k + 1) * P],
                                        identity=ident[:])
                    nc.scalar.copy(out=xT[:, k, :], in_=pst[:])
                ph2 = psD.tile([P, T2_USED], F32, tag="misc")
                for k in range(2):
                    nc.tensor.matmul(out=ph2[:], lhsT=xT[:, k, :], rhs=w2[:, k, :],
                                     start=(k == 0), stop=(k == 1))
                if t2s["tile"] is None:
                    tl_new = sb.tile([P, 4, T2_USED], F16, tag="t2s")
                    t2s["tile"] = tl_new
                    t2s["b0"] = b
                tl = t2s["tile"]
                nc.scalar.copy(out=tl[:, b - t2s["b0"], :], in_=ph2[:])
                nc.scalar.activation(out=adst2[:, b:b + 1], in_=ph2[:, 65:66],
                                     func=AF.Copy)
                if b - t2s["b0"] == 3 or b == NBLK - 1:
                    flush_t2(b + 1)
                # t2 collective chunks ride block completion.  Collectives
                # must issue on the Pool queue; trigger each chunk 2 blocks
                # after its input flush so the Pool SEQ wait (which would
                # block later gather issues) is already satisfied.
                for ci, (b0c, b1c) in enumerate(T2_CHUNKS):
                    if b == (min(b1c + 1, NBLK - 1) if b1c <= 36 else b1c - 1):
                        r0, r1 = b0c * P, min(b1c * P, NSH)
                        nc.gpsimd.collective_compute(
                            "AllGather", ALU.bypass,
                            ins=[t2_shard[r0:r1, :]],
                            outs=[t2_pack[NC * r0:NC * r1, :]],
                            replica_groups=[list(range(NC))])
                # early chunks' restrides run mid-L1 once their collectives
                # are long done; the rest go after L1
                if b == 30:
                    restride(0)
                if b == 38:
                    restride(1)
                if b == 46:
                    restride(2)

            edge_layer(t1_full, T1_ELEM, HIDDEN, HEADS, True, adst1, MC1,
                       finish1)
            for ci in range(3, len(T2_CHUNKS)):
                restride(ci)
            if DBG:
                nc.sync.dma_start(out=dbg_t2s[:], in_=t2_shard[0:256, :])
                nc.sync.dma_start(out=dbg_t2w[:], in_=t2_wide[0:256, :])

            # ---- layer 2 finisher ----
            o4 = {"tile": None, "b0": 0}

            def finish2(b, pacc):
                se = sb.tile([P, 1], F32, tag="se2")
                nc.vector.tensor_scalar_add(out=se[:], in0=pacc[:, REPR:MC2],
                                            scalar1=1e-16)
                rec = sb.tile([P, 1], F32, tag="rec2")
                nc.vector.reciprocal(out=rec[:], in_=se[:])
                if o4["tile"] is None:
                    ot_new = sb.tile([P, 4, REPR], F32, tag="o4")
                    o4["tile"] = ot_new
                    o4["b0"] = b
                tl = o4["tile"]
                nc.scalar.activation(out=tl[:, b - o4["b0"], :],
                                     in_=pacc[:, 0:REPR], func=AF.Copy,
                                     scale=rec[:, 0:1])
                if b - o4["b0"] == 3 or b == NBLK - 1:
                    b0 = o4["b0"]
                    rows = min((b + 1) * P, NSH) - b0 * P
                    if rows % P == 0:
                        nc.sync.dma_start(
                            out=out_t[b0 * P:b0 * P + rows, :]
                                .rearrange("(a p) e -> p a e", p=P),
                            in_=tl[:, 0:rows // P, :])
                    else:
                        full = rows // P
                        if full:
                            nc.sync.dma_start(
                                out=out_t[b0 * P:b0 * P + full * P, :]
                                    .rearrange("(a p) e -> p a e", p=P),
                                in_=tl[:, 0:full, :])
                        nc.sync.dma_start(
                            out=out_t[b0 * P + full * P:b0 * P + rows, :],
                            in_=tl[:rows - full * P, full, :])
                    o4["tile"] = None

            edge_layer(t2_wide, T2_ELEM, REPR, 1, False, adst2, MC2,
                       finish2)

    nc.finalize()
    globals()["LAST_NC"] = nc
    return nc


def kernel(**inputs):
    node_emb = np.asarray(inputs["node_emb"], np.float32)
    W1 = np.asarray(inputs["W1"], np.float32)
    att1_src = np.asarray(inputs["att1_src"], np.float32)
    att1_dst = np.asarray(inputs["att1_dst"], np.float32)
    b1 = np.asarray(inputs["b1"], np.float32)
    W2 = np.asarray(inputs["W2"], np.float32)
    att2_src = np.asarray(inputs["att2_src"], np.float32)
    att2_dst = np.asarray(inputs["att2_dst"], np.float32)
    b2 = np.asarray(inputs["b2"], np.float32)
    edge_index = np.asarray(inputs["edge_index"])
    assert not np.any(b1) and not np.any(b2), "nonzero biases unsupported"

    meta, idx_w, dl_T = _prep_edges(edge_index)

    # d-major permutation: table col d*8+h <- W1 output col h*32+d
    dd, hh = np.meshgrid(np.arange(OUT1), np.arange(HEADS), indexing="ij")
    src_cols = (hh * OUT1 + dd).reshape(-1)          # length 256, j=d*8+h
    A1s = np.einsum("ehd,hd->eh", W1.reshape(EMB, HEADS, OUT1), att1_src)
    A1d = np.einsum("ehd,hd->eh", W1.reshape(EMB, HEADS, OUT1), att1_dst)
    w1ext = np.concatenate([W1[:, src_cols], A1s, A1d], axis=1).astype(np.float16)
    A2s = W2 @ att2_src[0]
    A2d = W2 @ att2_dst[0]
    w2e_rows = np.concatenate([W2, A2s[:, None], A2d[:, None]], axis=1)
    w2ext = w2e_rows[src_cols, :].astype(np.float16)   # rows follow x16 cols
    # x16 = elu(v)+1, so GEMM2 adds a constant row correction -sum(W2ext rows)
    # injected via matmul(lhsT=ident, rhs=crow).
    crow = np.tile((-w2ext.astype(np.float32).sum(axis=0))[None, :],
                   (P, 1)).astype(np.float16)

    iota = np.tile(np.arange(P, dtype=np.float16), (P, 1))

    nc = _build(meta)

    embT = np.zeros((EMB, NROWS), np.float16)
    embT[:, :N] = node_emb.T.astype(np.float16)
    embm = np.zeros((NC, EMB, NBLK * P), np.float16)
    for c in range(NC):
        embm[c, :, :NSH] = node_emb[c * NSH:(c + 1) * NSH].T.astype(np.float16)

    in_maps = []
    for c in range(NC):
        in_maps.append({
            "embT": embT,
            "embTm": embm[c],
            "w1ext": w1ext,
            "w2ext": w2ext,
            "crow": crow,
            "idxw": idx_w[c],
            "dlT": dl_T[c],
            "iota": iota,
        })

    res = run_bass_kernel_spmd(nc, in_maps, core_ids=list(range(NC)))
    globals()["LAST_RES"] = res
    globals()["LAST_META"] = meta
    out = np.concatenate([res.results[c]["out"] for c in range(NC)], axis=0)
    return np.ascontiguousarray(out.astype(np.float32))


if __name__ == "__main__":
    rng = np.random.default_rng(0)
    ins = {
        "node_emb": rng.standard_normal((N, EMB), dtype=np.float32) * 0.05,
        "W1": rng.standard_normal((EMB, HIDDEN), dtype=np.float32) * 0.07,
        "att1_src": rng.standard_normal((HEADS, OUT1), dtype=np.float32) * 0.2,
        "att1_dst": rng.standard_normal((HEADS, OUT1), dtype=np.float32) * 0.2,
        "b1": np.zeros(HIDDEN, np.float32),
        "W2": rng.standard_normal((HIDDEN, REPR), dtype=np.float32) * 0.07,
        "att2_src": rng.standard_normal((1, REPR), dtype=np.float32) * 0.2,
        "att2_dst": rng.standard_normal((1, REPR), dtype=np.float32) * 0.2,
        "b2": np.zeros(REPR, np.float32),
        "edge_index": rng.integers(0, N, (2, E)).astype(np.int32),
    }
    out = kernel(**ins)
    print("out", out.shape, out.dtype, np.abs(out).mean())


# revision 61
# speedup vs baseline: 1.0533x; 1.0040x over previous
"""GAT (2-layer) on 8 Trainium2 NeuronCores — collective-minimal v2.

Strategy
--------
- Layer-1 node GEMM is REPLICATED: every core computes the full projected
  table t1_full[50176, 384] itself (rows: [h1 d-major 256 | a_src 8 | pad]),
  so no AllGather is needed before edge phase 1.  DMA-bound at ~112us,
  which beats the 364us modeled AllGather it replaces.
- Edges are routed to the core owning their destination (6250 nodes/core).
  Per dst block (128 nodes): two gather calls (src < s / src >= s so int16
  indices reach all 50176 rows), one-hot matrices via is_equal, transposed
  one-hots for the a_dst expansion, exp(leakyrelu(logits)) on Act written
  straight into the matmul rhs, messages = h1 * alpha with the d-major
  layout (DVE 2x mode), and a one-matmul-per-chunk scatter accumulation
  with the softmax denominators riding along as extra columns.
- h1 features are stored d-major ([d=32 x h=8] interleave) so the per-edge
  alpha broadcast lands on a middle axis and the multiply qualifies for the
  DVE 2x performance mode.
- Layer-2 table is collected PACKED (66 cols = 132B rows, 6.6MB vs 25.6MB)
  via a 5-chunk AllGather pipelined against layer-1 block production (small
  last chunk to shorten the tail), then restrided on-device into gatherable
  256B rows.  Collectives trigger 2 blocks after their input flush so the
  Pool SEQ never stalls gather issues; restrides are scheduled where their
  dependencies are already met so the in-order SP queue never blocks.
- The edge phase is software-pipelined 3 deep (one-hot prep at b+3, logit
  stage at b+1, message/scatter/finish at b) so the long cross-engine
  dependency chain overlaps across blocks despite in-order engine queues.
  All DMAs issue on the SP queue (an Activation-queue DMA issue blocks Act
  compute behind it).
"""

import math
import os

import numpy as np

import concourse.bass as bass
import concourse.mybir as mybir
import concourse.tile as tile
from concourse import bacc
from concourse.bass_utils import run_bass_kernel_spmd
from concourse.masks import make_identity

# problem constants (from the reference)
N = 50000
E = 500000
EMB = 128
HIDDEN = 256
HEADS = 8
OUT1 = 32
REPR = 64
NEG_SLOPE = 0.2

NC = 8
P = 128
NSH = N // NC                    # 6250 nodes per core
NBLK = (NSH + P - 1) // P        # 49 dst blocks per core
LASTB = NSH - (NBLK - 1) * P     # 106 nodes in last block
NROWS = 392 * P                  # 50176 padded table rows (identity node map)
HI_BASE = NROWS - 32768          # 17408: base row for the hi gather half

T1_ELEM = 384                    # fp16 row: h1(256 d-major) | a_src(8) | pad
T1_USED = 264
T2_ELEM = 128                    # fp16 row in t2_wide (256B gather stride)
T2_USED = 66                     # h2(64) | a2_src | a2_dst
MC1 = HIDDEN + HEADS             # 264 scatter cols layer 1
MC2 = REPR + 1                   # 65 scatter cols layer 2

F16 = mybir.dt.float16
F32 = mybir.dt.float32
I16 = mybir.dt.int16
AF = mybir.ActivationFunctionType
ALU = mybir.AluOpType

RING = 896                       # max idxs per gather call (1024-desc ring,
                                 # with the margin the ring needs in flight)
# t2 collective chunk boundaries (blocks; must be multiples of the 4-block
# staging flush so the shard rows are in DRAM before the collective reads them)
T2_CHUNKS = [(0, 12), (12, 24), (24, 36), (36, 44), (44, 48), (48, 49)]


def _ceil(a, b):
    return (a + b - 1) // b


def _prep_edges(edge_index):
    """Partition + sort edges; emit per-core idx/dl arrays + chunk metadata.

    Chunk metadata (kch_lo/kch_hi per block, call grouping) is shared across
    cores (max over cores) so the SPMD program is identical.
    """
    ei = np.asarray(edge_index)
    src = np.concatenate([ei[0], np.arange(N, dtype=np.int64)]).astype(np.int64)
    dst = np.concatenate([ei[1], np.arange(N, dtype=np.int64)]).astype(np.int64)
    core = dst // NSH
    dl = dst % NSH
    blk = dl // P
    dl128 = (dl % P).astype(np.float16)

    # counts per (core, block, half) for a candidate split s
    order0 = np.lexsort((src, blk, core))
    sc, bc, srt, dlc = core[order0], blk[order0], src[order0], dl128[order0]

    def counts_for(s):
        hi = (srt >= s).astype(np.int64)
        key = (sc * NBLK + bc) * 2 + hi
        return np.bincount(key, minlength=NC * NBLK * 2).reshape(NC, NBLK, 2)

    best = None
    for s in range(HI_BASE + 256, 32769, 256):
        c = counts_for(s)
        kl = _ceil(c[:, :, 0].max(axis=0), P)
        kh = _ceil(c[:, :, 1].max(axis=0), P)
        if kl.max() > 7 or kh.max() > 7:
            continue              # stay under the 1024-desc ring with margin
        tot = int(kl.sum() + kh.sum())
        if best is None or tot < best[0]:
            best = (tot, s, kl, kh)
    assert best is not None
    _, split_s, kch_lo, kch_hi = best
    kch_lo = kch_lo.astype(int)
    kch_hi = kch_hi.astype(int)

    # gather call grouping: pairs of blocks per half, <= RING idxs per call
    calls = []                    # flat list of (half, [blocks])
    pair_calls = []               # per pair: list of indices into `calls`
    for q in range(_ceil(NBLK, 2)):
        bs = [b for b in (2 * q, 2 * q + 1) if b < NBLK]
        mine = []
        for half, kch in ((0, kch_lo), (1, kch_hi)):
            tot = sum(int(kch[b]) for b in bs)
            if tot * P <= RING:
                mine.append(len(calls))
                calls.append((half, bs))
            else:
                for b in bs:
                    assert kch[b] * P <= RING
                    mine.append(len(calls))
                    calls.append((half, [b]))
        pair_calls.append(mine)

    cpb = kch_lo + kch_hi                       # chunks per block
    nchunk = int(cpb.sum())
    chunk_col = np.zeros(NBLK + 1, int)
    np.cumsum(cpb, out=chunk_col[1:])           # dl_T column of block b
    call_cols = []                              # idx col offset per call
    off = 0
    for half, bs in calls:
        call_cols.append(off)
        off += sum(int((kch_lo if half == 0 else kch_hi)[b]) for b in bs) * 8
    idx_cols = off

    idx_w = np.zeros((NC, 128, idx_cols), np.int16)
    dl_T = np.full((NC, 128, nchunk), 200.0, np.float16)
    for c in range(NC):
        m = core == c
        s_, d_, b_ = src[m], dl128[m], blk[m]
        hi_ = (s_ >= split_s)
        o = np.lexsort((s_, hi_.astype(np.int64), b_))
        s_, d_, b_, hi_ = s_[o], d_[o], b_[o], hi_[o]
        key = b_ * 2 + hi_
        cnt = np.bincount(key, minlength=NBLK * 2)
        starts = np.zeros(NBLK * 2 + 1, np.int64)
        np.cumsum(cnt, out=starts[1:])
        # per (block, half): idx values and dl columns
        for ci, (half, bs) in enumerate(calls):
            col0 = call_cols[ci]
            w = []
            for b in bs:
                kch = int((kch_lo if half == 0 else kch_hi)[b])
                g = b * 2 + half
                n = int(cnt[g])
                sl = slice(starts[g], starts[g] + n)
                iv = np.zeros(kch * P, np.int16)
                iv[:n] = (s_[sl] - (HI_BASE if half else 0)).astype(np.int16)
                w.append(iv)
                dv = np.full(kch * P, 200.0, np.float16)
                dv[:n] = d_[sl]
                cc = chunk_col[b] + (0 if half == 0 else int(kch_lo[b]))
                dl_T[c, :, cc:cc + kch] = dv.reshape(kch, P).T
            wv = np.concatenate(w)
            ww = wv.reshape(len(wv) // 16, 16).T       # wrap [i%16, i//16]
            idx_w[c, :, col0:col0 + ww.shape[1]] = np.tile(ww, (8, 1))
    meta = dict(split_s=split_s, kch_lo=kch_lo, kch_hi=kch_hi, calls=calls,
                pair_calls=pair_calls, call_cols=call_cols,
                chunk_col=chunk_col, idx_cols=idx_cols, nchunk=nchunk)
    return meta, idx_w, dl_T


def _build(meta):
    kch_lo, kch_hi = meta["kch_lo"], meta["kch_hi"]
    calls, call_cols = meta["calls"], meta["call_cols"]
    pair_calls = meta["pair_calls"]
    chunk_col = meta["chunk_col"]
    nchunk, idx_cols = meta["nchunk"], meta["idx_cols"]
    cpb = kch_lo + kch_hi
    CPBMAX = int(cpb.max())
    GMAX = max(sum(int((kch_lo if h == 0 else kch_hi)[b]) for b in bs)
               for h, bs in calls)

    nc = bacc.Bacc(None, target_bir_lowering=False)

    # ---- inputs (per core) ----
    embT_in = nc.dram_tensor("embT", [EMB, NROWS], F16, kind="ExternalInput")
    embm_in = nc.dram_tensor("embTm", [EMB, NBLK * P], F16, kind="ExternalInput")
    w1_in = nc.dram_tensor("w1ext", [EMB, 272], F16, kind="ExternalInput")
    w2_in = nc.dram_tensor("w2ext", [HIDDEN, T2_USED], F16, kind="ExternalInput")
    crow_in = nc.dram_tensor("crow", [P, T2_USED], F16, kind="ExternalInput")
    idx_in = nc.dram_tensor("idxw", [128, idx_cols], I16, kind="ExternalInput")
    dl_in = nc.dram_tensor("dlT", [128, nchunk], F16, kind="ExternalInput")
    iota_in = nc.dram_tensor("iota", [P, P], F16, kind="ExternalInput")
    out_t = nc.dram_tensor("out", [NSH, REPR], F32, kind="ExternalOutput")
    DBG = bool(os.environ.get("KDBG"))
    if DBG:
        dbg_t1 = nc.dram_tensor("dbg_t1", [256, T1_ELEM], F16, kind="ExternalOutput")
        dbg_adst1 = nc.dram_tensor("dbg_adst1", [P, NBLK * HEADS], F16, kind="ExternalOutput")
        dbg_pacc = nc.dram_tensor("dbg_pacc", [P, MC1], F32, kind="ExternalOutput")
        dbg_x16 = nc.dram_tensor("dbg_x16", [P, HIDDEN], F16, kind="ExternalOutput")
        dbg_t2s = nc.dram_tensor("dbg_t2s", [256, T2_USED], F16, kind="ExternalOutput")
        dbg_t2w = nc.dram_tensor("dbg_t2w", [256, T2_ELEM], F16, kind="ExternalOutput")

    # ---- internal DRAM ----
    t1_full = nc.dram_tensor("t1_full", [NROWS, T1_ELEM], F16, kind="Internal")
    t2_shard = nc.dram_tensor("t2_shard", [NSH, T2_USED], F16, kind="Internal")
    t2_pack = nc.dram_tensor("t2_pack", [N, T2_USED], F16, kind="Internal",
                             addr_space="Shared")
    t2_wide = nc.dram_tensor("t2_wide", [NROWS, T2_ELEM], F16, kind="Internal")

    with tile.TileContext(nc) as tc:
        with (
            tc.tile_pool(name="const", bufs=1) as cst,
            tc.tile_pool(name="sb", bufs=3) as sb,
            tc.tile_pool(name="gp", bufs=5) as gp,
            tc.tile_pool(name="rp", bufs=4) as rp,
            tc.tile_pool(name="sb2", bufs=3) as sb2,
            tc.tile_pool(name="oh", bufs=4) as ohp,
            tc.tile_pool(name="ohT", bufs=4) as ohtp,
            tc.tile_pool(name="psA", bufs=3, space="PSUM") as psA,
            tc.tile_pool(name="psB", bufs=2, space="PSUM") as psB,
            tc.tile_pool(name="psC", bufs=1, space="PSUM") as psC,
            tc.tile_pool(name="psD", bufs=2, space="PSUM") as psD,
        ):
            # ---- constants ----
            iota = cst.tile([P, P], F16)
            nc.sync.dma_start(out=iota[:], in_=iota_in[:])
            ident = cst.tile([P, P], F16)
            make_identity(nc, ident[:])
            w1 = cst.tile([EMB, 272], F16)
            nc.sync.dma_start(out=w1[:], in_=w1_in[:])
            w2 = cst.tile([P, 2, T2_USED], F16)
            nc.sync.dma_start(out=w2[:, 0, :], in_=w2_in[0:P, :])
            nc.sync.dma_start(out=w2[:, 1, :], in_=w2_in[P:HIDDEN, :])
            crow = cst.tile([P, T2_USED], F16)
            nc.sync.dma_start(out=crow[:], in_=crow_in[:])
            it_all = cst.tile([128, idx_cols], I16)
            nc.sync.dma_start(out=it_all[:], in_=idx_in[:])
            dl_all = cst.tile([128, nchunk], F16)
            nc.sync.dma_start(out=dl_all[:], in_=dl_in[:])
            adst1 = cst.tile([P, NBLK * HEADS], F16)
            adst2 = cst.tile([P, NBLK], F16)

            # ---- a_dst mini-GEMM over my own dst shard ----
            for g in range(7):
                em = sb.tile([EMB, 7, P], F16, tag="embm")
                nc.sync.dma_start(
                    out=em[:].rearrange("p a n -> p (a n)"),
                    in_=embm_in[:, g * 7 * P:(g + 1) * 7 * P])
                psx = psC.tile([P, 7, HEADS], F32, tag="pse")
                for j in range(7):
                    nc.tensor.matmul(out=psx[:, j, :], lhsT=em[:, j, :],
                                     rhs=w1[:, 264:272], start=True, stop=True)
                nc.vector.tensor_copy(
                    out=adst1[:, g * 56:(g + 1) * 56],
                    in_=psx[:].rearrange("p a h -> p (a h)"))

            # ---- phase A: replicated full-table GEMM1 (8 blocks/group) ----
            etq = {}

            def load_embT(g):
                et = sb.tile([EMB, 8, P], F16, tag="embT")
                nc.sync.dma_start(
                    out=et[:].rearrange("p a n -> p (a n)"),
                    in_=embT_in[:, g * 8 * P:(g + 1) * 8 * P])
                etq[g] = et

            load_embT(0)
            load_embT(1)
            for g in range(49):
                if g + 2 < 49:
                    load_embT(g + 2)
                et = etq.pop(g)
                t1s = sb.tile([P, 8, T1_USED], F16, tag="t1s")
                for j in range(8):
                    ph = psA.tile([P, T1_USED], F32, tag="acc")
                    nc.tensor.matmul(out=ph[:], lhsT=et[:, j, :],
                                     rhs=w1[:, 0:T1_USED], start=True, stop=True)
                    eng = (nc.vector, nc.scalar, nc.vector)[j % 3]
                    if eng is nc.scalar:
                        eng.copy(out=t1s[:, j, :], in_=ph[:])
                    else:
                        eng.tensor_copy(out=t1s[:, j, :], in_=ph[:])
                nc.sync.dma_start(
                    out=t1_full[g * 8 * P:(g + 1) * 8 * P, 0:T1_USED]
                        .rearrange("(a p) e -> p a e", p=P),
                    in_=t1s[:])

            if DBG:
                nc.sync.dma_start(out=dbg_t1[:], in_=t1_full[0:256, :])
                nc.sync.dma_start(out=dbg_adst1[:], in_=adst1[:])

            # ---- edge phase helper ----
            def edge_layer(t_full, elem, hid, heads, dmajor, adst_t, mcols,
                           out_cb):
                PREQ = 1              # pair prefetch distance
                gq = {}

                def issue_gathers(q):
                    gs = {}
                    for ci in pair_calls[q]:
                        half, bs = calls[ci]
                        kch = kch_lo if half == 0 else kch_hi
                        ch = sum(int(kch[b]) for b in bs)
                        gt = gp.tile([P, GMAX * T1_ELEM], F16, tag=f"g{half}")
                        col0 = call_cols[ci]
                        r0 = HI_BASE if half else 0
                        nc.gpsimd.dma_gather(
                            out_ap=gt[:, 0:ch * elem]
                                .rearrange("p (t e) -> p t e", e=elem),
                            in_ap=t_full[r0:NROWS, :],
                            idxs_ap=it_all[:, col0:col0 + ch * 8],
                            num_idxs=ch * P, num_idxs_reg=ch * P,
                            elem_size=elem)
                        off = 0
                        for b in bs:
                            gs[(b, half)] = (gt, off)
                            off += int(kch[b])
                    gq[q] = gs

                npairs = len(pair_calls)
                nextq = [0]

                def ensure_pairs(upto):
                    while nextq[0] <= min(upto, npairs - 1):
                        issue_gathers(nextq[0])
                        nextq[0] += 1

                def gv(b, half, c0, c1, e0, e1):
                    gt, off = gq[b // 2][(b, half)]
                    v = gt[:, 0:(off + c1) * elem].rearrange(
                        "p (t e) -> p t e", e=elem)
                    return v[:, off + c0:off + c1, e0:e1]

                state = {}

                def stage0(b):
                    """one-hots + transposed one-hots (needs only dl/iota)."""
                    t = int(cpb[b])
                    cc = int(chunk_col[b])
                    oh = ohp.tile([P, CPBMAX, P], F16, tag="oh")
                    nc.vector.tensor_tensor(
                        out=oh[:, 0:t, :],
                        in0=dl_all[:, cc:cc + t, None].to_broadcast([P, t, P]),
                        in1=iota[:, None, :].to_broadcast([P, t, P]),
                        op=ALU.is_equal)
                    ohT = ohtp.tile([P, CPBMAX, P], F16, tag="ohT")
                    for g0 in range(0, t, 8):
                        g1 = min(g0 + 8, t)
                        pst = psB.tile([P, 8, P], F16, tag="ohT")
                        for k in range(g0, g1):
                            nc.tensor.transpose(out=pst[:, k - g0, :],
                                                in_=oh[:, k, :],
                                                identity=ident[:])
                        eng = nc.scalar
                        if eng is nc.scalar:
                            eng.copy(
                                out=ohT[:, g0:g1, :].rearrange("p t n -> p (t n)"),
                                in_=pst[:, 0:g1 - g0, :]
                                    .rearrange("p t n -> p (t n)"))
                        else:
                            eng.tensor_copy(
                                out=ohT[:, g0:g1, :].rearrange("p t n -> p (t n)"),
                                in_=pst[:, 0:g1 - g0, :]
                                    .rearrange("p t n -> p (t n)"))
                    state[b] = dict(oh=oh, ohT=ohT)

                def stage1(b):
                    """logits -> exp weights (needs gathers + ohT)."""
                    st = state[b]
                    klo, khi = int(kch_lo[b]), int(kch_hi[b])
                    t = klo + khi
                    ohT = st["ohT"]
                    pse = psC.tile([P, CPBMAX * heads], F32, tag="pse")
                    for k in range(t):
                        half, ko = (0, k) if k < klo else (1, k - klo)
                        nc.tensor.matmul(
                            out=pse[:, k * heads:(k + 1) * heads],
                            lhsT=ohT[:, k, :],
                            rhs=adst_t[:, b * heads:(b + 1) * heads],
                            start=True, stop=False)
                        nc.tensor.matmul(
                            out=pse[:, k * heads:(k + 1) * heads],
                            lhsT=ident[:],
                            rhs=gv(b, half, ko, ko + 1, hid, hid + heads)[:, 0, :],
                            start=False, stop=True)
                    lk = sb2.tile([P, CPBMAX * heads], F32, tag=f"lk{hid}")
                    nc.scalar.activation(
                        out=lk[:, 0:t * heads],
                        in_=pse[:, 0:t * heads],
                        func=AF.Prelu, alpha=NEG_SLOPE)
                    rhs = rp.tile([P, CPBMAX, mcols], F16, tag="rhs")
                    nc.scalar.activation(
                        out=rhs[:, 0:t, hid:hid + heads],
                        in_=lk[:, 0:t * heads]
                            .rearrange("p (t h) -> p t h", h=heads),
                        func=AF.Exp)
                    st["rhs"] = rhs

                def stage2(b):
                    """messages + scatter + finish."""
                    st = state.pop(b)
                    klo, khi = int(kch_lo[b]), int(kch_hi[b])
                    t = klo + khi
                    oh, rhs = st["oh"], st["rhs"]
                    for half, k0, k1 in ((0, 0, klo), (1, klo, t)):
                        kk = k1 - k0
                        eng = nc.vector
                        if dmajor:
                            eng.tensor_tensor(
                                out=rhs[:, k0:k1, 0:hid]
                                    .rearrange("p t (d h) -> p t d h", h=heads),
                                in0=gv(b, half, 0, kk, 0, hid)
                                    .rearrange("p t (d h) -> p t d h", h=heads),
                                in1=rhs[:, k0:k1, None, hid:hid + heads]
                                    .to_broadcast([P, kk, hid // heads, heads]),
                                op=ALU.mult)
                        else:
                            eng.tensor_tensor(
                                out=rhs[:, k0:k1, 0:hid],
                                in0=gv(b, half, 0, kk, 0, hid),
                                in1=rhs[:, k0:k1, hid:hid + 1]
                                    .to_broadcast([P, kk, hid]),
                                op=ALU.mult)
                    pacc = psA.tile([P, mcols], F32, tag="acc")
                    for k in range(t):
                        nc.tensor.matmul(out=pacc[:], lhsT=oh[:, k, :],
                                         rhs=rhs[:, k, :], start=(k == 0),
                                         stop=(k == t - 1))
                    if b % 2 == 1:
                        gq.pop(b // 2)
                    out_cb(b, pacc)

                ensure_pairs(1)
                stage0(0)
                stage0(1)
                stage0(2)
                stage1(0)
                for b in range(NBLK):
                    ensure_pairs((b + 5) // 2)
                    if b + 3 < NBLK:
                        stage0(b + 3)
                    if b + 1 < NBLK:
                        stage1(b + 1)
                    stage2(b)

            # ---- layer 1 finisher ----
            def restride(ci):
                b0c, b1c = T2_CHUNKS[ci]
                r0, r1 = b0c * P, min(b1c * P, NSH)
                for c in range(NC):
                    nc.sync.dma_start(
                        out=t2_wide[c * NSH + r0:c * NSH + r1, 0:T2_USED],
                        in_=t2_pack[NC * r0 + c * (r1 - r0):
                                    NC * r0 + (c + 1) * (r1 - r0), :])

            t2s = {"tile": None, "b0": 0}

            def flush_t2(bend):
                tl, b0 = t2s["tile"], t2s["b0"]
                if tl is None:
                    return
                rows = min(bend * P, NSH) - b0 * P
                nc.sync.dma_start(
                    out=t2_shard[b0 * P:b0 * P + rows, :]
                        .rearrange("(a p) e -> p a e", p=P)
                    if rows % P == 0 else
                    t2_shard[b0 * P:b0 * P + rows, :],
                    in_=tl[:, 0:rows // P, :] if rows % P == 0
                    else tl[:rows, bend - 1 - b0, :])
                t2s["tile"] = None

            def finish1(b, pacc):
                if DBG and b == 0:
                    dpc = sb.tile([P, MC1], F32, tag="dpc")
                    nc.vector.tensor_copy(out=dpc[:], in_=pacc[:])
                    nc.sync.dma_start(out=dbg_pacc[:], in_=dpc[:])
                se = sb.tile([P, HEADS], F32, tag="se")
                nc.scalar.activation(out=se[:], in_=pacc[:, HIDDEN:MC1],
                                     func=AF.Copy, bias=1e-16)
                rec = sb.tile([P, HEADS], F32, tag="rec")
                nc.vector.reciprocal(out=rec[:], in_=se[:])
                v = sb.tile([P, HIDDEN], F32, tag="v")
                nc.vector.tensor_tensor(
                    out=v[:].rearrange("p (d h) -> p d h", h=HEADS),
                    in0=pacc[:, 0:HIDDEN].rearrange("p (d h) -> p d h", h=HEADS),
                    in1=rec[:, None, :].to_broadcast([P, OUT1, HEADS]),
                    op=ALU.mult)
                # elu(v) = relu(v) + exp(min(v,0)) - 1
                r = sb.tile([P, HIDDEN], F32, tag="relu")
                nc.scalar.activation(out=r[:], in_=v[:], func=AF.Relu)
                mn = sb.tile([P, HIDDEN], F32, tag="mn")
                nc.scalar.activation(out=mn[:], in_=v[:], func=AF.Relu,
                                     scale=-1.0)
                em = sb.tile([P, HIDDEN], F32, tag="em")
                nc.scalar.activation(out=em[:], in_=mn[:], func=AF.Exp,
                                     scale=-1.0)
                x = sb.tile([P, HIDDEN], F32, tag="x")
                nc.vector.tensor_tensor(out=x[:], in0=r[:], in1=em[:],
                                        op=ALU.add)
                x16 = sb.tile([P, HIDDEN], F16, tag="x16")
                nc.scalar.activation(out=x16[:], in_=x[:], func=AF.Copy,
                                     bias=-1.0)
                if DBG and b == 0:
                    nc.sync.dma_start(out=dbg_x16[:], in_=x16[:])
                xT = sb.tile([P, 2, P], F16, tag="xT")
                for k in range(2):
                    pst = psD.tile([P, P], F16, tag="misc")
                    nc.tensor.transpose(out=pst[:], in_=x16[:, k * P:(k + 1) * P],
                                        identity=ident[:])
                    nc.scalar.copy(out=xT[:, k, :], in_=pst[:])
                ph2 = psD.tile([P, T2_USED], F32, tag="misc")
                for k in range(2):
                    nc.tensor.matmul(out=ph2[:], lhsT=xT[:, k, :], rhs=w2[:, k, :],
                                     start=(k == 0), stop=(k == 1))
                if t2s["tile"] is None:
                    tl_new = sb.tile([P, 4, T2_USED], F16, tag="t2s")
                    t2s["tile"] = tl_new
                    t2s["b0"] = b
                tl = t2s["tile"]
                nc.scalar.copy(out=tl[:, b - t2s["b0"], :], in_=ph2[:])
                nc.scalar.activation(out=adst2[:, b:b + 1], in_=ph2[:, 65:66],
                                     func=AF.Copy)
                if b - t2s["b0"] == 3 or b == NBLK - 1:
                    flush_t2(b + 1)
                # t2 collective chunks ride block completion.  Collectives
                # must issue on the Pool queue; trigger each chunk 2 blocks
                # after its input flush so the Pool SEQ wait (which would
                # block later gather issues) is already satisfied.
                for ci, (b0c, b1c) in enumerate(T2_CHUNKS):
                    if b == (min(b1c + 1, NBLK - 1) if b1c <= 36 else b1c - 1):
                        r0, r1 = b0c * P, min(b1c * P, NSH)
                        nc.gpsimd.collective_compute(
                            "AllGather", ALU.bypass,
                            ins=[t2_shard[r0:r1, :]],
                            outs=[t2_pack[NC * r0:NC * r1, :]],
                            replica_groups=[list(range(NC))])
                # early chunks' restrides run mid-L1 once their collectives
                # are long done; the rest go after L1
                if b == 30:
                    restride(0)
                if b == 38:
                    restride(1)
                if b == 46:
                    restride(2)

            edge_layer(t1_full, T1_ELEM, HIDDEN, HEADS, True, adst1, MC1,
                       finish1)
            for ci in range(3, len(T2_CHUNKS)):
                restride(ci)
            if DBG:
                nc.sync.dma_start(out=dbg_t2s[:], in_=t2_shard[0:256, :])
                nc.sync.dma_start(out=dbg_t2w[:], in_=t2_wide[0:256, :])

            # ---- layer 2 finisher ----
            o4 = {"tile": None, "b0": 0}

            def finish2(b, pacc):
                se = sb.tile([P, 1], F32, tag="se2")
                nc.vector.tensor_scalar_add(out=se[:], in0=pacc[:, REPR:MC2],
                                            scalar1=1e-16)
                rec = sb.tile([P, 1], F32, tag="rec2")
                nc.vector.reciprocal(out=rec[:], in_=se[:])
                if o4["tile"] is None:
                    ot_new = sb.tile([P, 4, REPR], F32, tag="o4")
                    o4["tile"] = ot_new
                    o4["b0"] = b
                tl = o4["tile"]
                nc.scalar.activation(out=tl[:, b - o4["b0"], :],
                                     in_=pacc[:, 0:REPR], func=AF.Copy,
                                     scale=rec[:, 0:1])
                if b - o4["b0"] == 3 or b == NBLK - 1:
                    b0 = o4["b0"]
                    rows = min((b + 1) * P, NSH) - b0 * P
                    if rows % P == 0:
                        nc.sync.dma_start(
                            out=out_t[b0 * P:b0 * P + rows, :]
                                .rearrange("(a p) e -> p a e", p=P),
                            in_=tl[:, 0:rows // P, :])
                    else:
                        full = rows // P
                        if full:
                            nc.sync.dma_start(
                                out=out_t[b0 * P:b0 * P + full * P, :]
                                    .rearrange("(a p) e -> p a e", p=P),
                                in_=tl[:, 0:full, :])
                        nc.sync.dma_start(
                            out=out_t[b0 * P + full * P:b0 * P + rows, :],
                            in_=tl[:rows - full * P, full, :])
                    o4["tile"] = None

            edge_layer(t2_wide, T2_ELEM, REPR, 1, False, adst2, MC2,
                       finish2)

    nc.finalize()
    globals()["LAST_NC"] = nc
    return nc


def kernel(**inputs):
    node_emb = np.asarray(inputs["node_emb"], np.float32)
    W1 = np.asarray(inputs["W1"], np.float32)
    att1_src = np.asarray(inputs["att1_src"], np.float32)
    att1_dst = np.asarray(inputs["att1_dst"], np.float32)
    b1 = np.asarray(inputs["b1"], np.float32)
    W2 = np.asarray(inputs["W2"], np.float32)
    att2_src = np.asarray(inputs["att2_src"], np.float32)
    att2_dst = np.asarray(inputs["att2_dst"], np.float32)
    b2 = np.asarray(inputs["b2"], np.float32)
    edge_index = np.asarray(inputs["edge_index"])
    assert not np.any(b1) and not np.any(b2), "nonzero biases unsupported"

    meta, idx_w, dl_T = _prep_edges(edge_index)

    # d-major permutation: table col d*8+h <- W1 output col h*32+d
    dd, hh = np.meshgrid(np.arange(OUT1), np.arange(HEADS), indexing="ij")
    src_cols = (hh * OUT1 + dd).reshape(-1)          # length 256, j=d*8+h
    A1s = np.einsum("ehd,hd->eh", W1.reshape(EMB, HEADS, OUT1), att1_src)
    A1d = np.einsum("ehd,hd->eh", W1.reshape(EMB, HEADS, OUT1), att1_dst)
    w1ext = np.concatenate([W1[:, src_cols], A1s, A1d], axis=1).astype(np.float16)
    A2s = W2 @ att2_src[0]
    A2d = W2 @ att2_dst[0]
    w2e_rows = np.concatenate([W2, A2s[:, None], A2d[:, None]], axis=1)
    w2ext = w2e_rows[src_cols, :].astype(np.float16)   # rows follow x16 cols
    # x16 = elu(v)+1, so GEMM2 adds a constant row correction -sum(W2ext rows)
    # injected via matmul(lhsT=ident, rhs=crow).
    crow = np.tile((-w2ext.astype(np.float32).sum(axis=0))[None, :],
                   (P, 1)).astype(np.float16)

    iota = np.tile(np.arange(P, dtype=np.float16), (P, 1))

    nc = _build(meta)

    embT = np.zeros((EMB, NROWS), np.float16)
    embT[:, :N] = node_emb.T.astype(np.float16)
    embm = np.zeros((NC, EMB, NBLK * P), np.float16)
    for c in range(NC):
        embm[c, :, :NSH] = node_emb[c * NSH:(c + 1) * NSH].T.astype(np.float16)

    in_maps = []
    for c in range(NC):
        in_maps.append({
            "embT": embT,
            "embTm": embm[c],
            "w1ext": w1ext,
            "w2ext": w2ext,
            "crow": crow,
            "idxw": idx_w[c],
            "dlT": dl_T[c],
            "iota": iota,
        })

    res = run_bass_kernel_spmd(nc, in_maps, core_ids=list(range(NC)))
    globals()["LAST_RES"] = res
    globals()["LAST_META"] = meta
    out = np.concatenate([res.results[c]["out"] for c in range(NC)], axis=0)
    return np.ascontiguousarray(out.astype(np.float32))


if __name__ == "__main__":
    rng = np.random.default_rng(0)
    ins = {
        "node_emb": rng.standard_normal((N, EMB), dtype=np.float32) * 0.05,
        "W1": rng.standard_normal((EMB, HIDDEN), dtype=np.float32) * 0.07,
        "att1_src": rng.standard_normal((HEADS, OUT1), dtype=np.float32) * 0.2,
        "att1_dst": rng.standard_normal((HEADS, OUT1), dtype=np.float32) * 0.2,
        "b1": np.zeros(HIDDEN, np.float32),
        "W2": rng.standard_normal((HIDDEN, REPR), dtype=np.float32) * 0.07,
        "att2_src": rng.standard_normal((1, REPR), dtype=np.float32) * 0.2,
        "att2_dst": rng.standard_normal((1, REPR), dtype=np.float32) * 0.2,
        "b2": np.zeros(REPR, np.float32),
        "edge_index": rng.integers(0, N, (2, E)).astype(np.int32),
    }
    out = kernel(**ins)
    print("out", out.shape, out.dtype, np.abs(out).mean())


# revision 62
# speedup vs baseline: 1.0638x; 1.0100x over previous
"""GAT (2-layer) on 8 Trainium2 NeuronCores — collective-minimal v2.

Strategy
--------
- Layer-1 node GEMM is REPLICATED: every core computes the full projected
  table t1_full[50176, 384] itself (rows: [h1 d-major 256 | a_src 8 | pad]),
  so no AllGather is needed before edge phase 1.  DMA-bound at ~112us,
  which beats the 364us modeled AllGather it replaces.
- Edges are routed to the core owning their destination (6250 nodes/core).
  Per dst block (128 nodes): two gather calls (src < s / src >= s so int16
  indices reach all 50176 rows), one-hot matrices via is_equal, transposed
  one-hots for the a_dst expansion, exp(leakyrelu(logits)) on Act written
  straight into the matmul rhs, messages = h1 * alpha with the d-major
  layout (DVE 2x mode), and a one-matmul-per-chunk scatter accumulation
  with the softmax denominators riding along as extra columns.
- h1 features are stored d-major ([d=32 x h=8] interleave) so the per-edge
  alpha broadcast lands on a middle axis and the multiply qualifies for the
  DVE 2x performance mode.
- Layer-2 table is collected PACKED (66 cols = 132B rows, 6.6MB vs 25.6MB)
  via a 5-chunk AllGather pipelined against layer-1 block production (small
  last chunk to shorten the tail), then restrided on-device into gatherable
  256B rows.  Collectives trigger 2 blocks after their input flush so the
  Pool SEQ never stalls gather issues; restrides are scheduled where their
  dependencies are already met so the in-order SP queue never blocks.
- The edge phase is software-pipelined 3 deep (one-hot prep at b+3, logit
  stage at b+1, message/scatter/finish at b) so the long cross-engine
  dependency chain overlaps across blocks despite in-order engine queues.
  All DMAs issue on the SP queue (an Activation-queue DMA issue blocks Act
  compute behind it).
"""

import math
import os

import numpy as np

import concourse.bass as bass
import concourse.mybir as mybir
import concourse.tile as tile
from concourse import bacc
from concourse.bass_utils import run_bass_kernel_spmd
from concourse.masks import make_identity

# problem constants (from the reference)
N = 50000
E = 500000
EMB = 128
HIDDEN = 256
HEADS = 8
OUT1 = 32
REPR = 64
NEG_SLOPE = 0.2

NC = 8
P = 128
NSH = N // NC                    # 6250 nodes per core
NBLK = (NSH + P - 1) // P        # 49 dst blocks per core
LASTB = NSH - (NBLK - 1) * P     # 106 nodes in last block
NROWS = 392 * P                  # 50176 padded table rows (identity node map)
HI_BASE = NROWS - 32768          # 17408: base row for the hi gather half

T1_ELEM = 384                    # fp16 row: h1(256 d-major) | a_src(8) | pad
T1_USED = 264
T2_ELEM = 128                    # fp16 row in t2_wide (256B gather stride)
T2_USED = 66                     # h2(64) | a2_src | a2_dst
MC1 = HIDDEN + HEADS             # 264 scatter cols layer 1
MC2 = REPR + 1                   # 65 scatter cols layer 2

F16 = mybir.dt.float16
F32 = mybir.dt.float32
I16 = mybir.dt.int16
AF = mybir.ActivationFunctionType
ALU = mybir.AluOpType

RING = 896                       # max idxs per gather call (1024-desc ring,
                                 # with the margin the ring needs in flight)
# t2 collective chunk boundaries (blocks; must be multiples of the 4-block
# staging flush so the shard rows are in DRAM before the collective reads them)
T2_CHUNKS = [(0, 12), (12, 24), (24, 36), (36, 44), (44, 48), (48, 49)]


def _ceil(a, b):
    return (a + b - 1) // b


def _prep_edges(edge_index):
    """Partition + sort edges; emit per-core idx/dl arrays + chunk metadata.

    Chunk metadata (kch_lo/kch_hi per block, call grouping) is shared across
    cores (max over cores) so the SPMD program is identical.
    """
    ei = np.asarray(edge_index)
    src = np.concatenate([ei[0], np.arange(N, dtype=np.int64)]).astype(np.int64)
    dst = np.concatenate([ei[1], np.arange(N, dtype=np.int64)]).astype(np.int64)
    core = dst // NSH
    dl = dst % NSH
    blk = dl // P
    dl128 = (dl % P).astype(np.float16)

    # counts per (core, block, half) for a candidate split s
    order0 = np.lexsort((src, blk, core))
    sc, bc, srt, dlc = core[order0], blk[order0], src[order0], dl128[order0]

    def counts_for(s):
        hi = (srt >= s).astype(np.int64)
        key = (sc * NBLK + bc) * 2 + hi
        return np.bincount(key, minlength=NC * NBLK * 2).reshape(NC, NBLK, 2)

    best = None
    for s in range(HI_BASE + 256, 32769, 256):
        c = counts_for(s)
        kl = _ceil(c[:, :, 0].max(axis=0), P)
        kh = _ceil(c[:, :, 1].max(axis=0), P)
        if kl.max() > 7 or kh.max() > 7:
            continue              # stay under the 1024-desc ring with margin
        tot = int(kl.sum() + kh.sum())
        if best is None or tot < best[0]:
            best = (tot, s, kl, kh)
    assert best is not None
    _, split_s, kch_lo, kch_hi = best
    kch_lo = kch_lo.astype(int)
    kch_hi = kch_hi.astype(int)

    # gather call grouping: pairs of blocks per half, <= RING idxs per call
    calls = []                    # flat list of (half, [blocks])
    pair_calls = []               # per pair: list of indices into `calls`
    for q in range(_ceil(NBLK, 2)):
        bs = [b for b in (2 * q, 2 * q + 1) if b < NBLK]
        mine = []
        for half, kch in ((0, kch_lo), (1, kch_hi)):
            tot = sum(int(kch[b]) for b in bs)
            if tot * P <= RING:
                mine.append(len(calls))
                calls.append((half, bs))
            else:
                for b in bs:
                    assert kch[b] * P <= RING
                    mine.append(len(calls))
                    calls.append((half, [b]))
        pair_calls.append(mine)

    cpb = kch_lo + kch_hi                       # chunks per block
    nchunk = int(cpb.sum())
    chunk_col = np.zeros(NBLK + 1, int)
    np.cumsum(cpb, out=chunk_col[1:])           # dl_T column of block b
    call_cols = []                              # idx col offset per call
    off = 0
    for half, bs in calls:
        call_cols.append(off)
        off += sum(int((kch_lo if half == 0 else kch_hi)[b]) for b in bs) * 8
    idx_cols = off

    idx_w = np.zeros((NC, 128, idx_cols), np.int16)
    dl_T = np.full((NC, 128, nchunk), 200.0, np.float16)
    for c in range(NC):
        m = core == c
        s_, d_, b_ = src[m], dl128[m], blk[m]
        hi_ = (s_ >= split_s)
        o = np.lexsort((s_, hi_.astype(np.int64), b_))
        s_, d_, b_, hi_ = s_[o], d_[o], b_[o], hi_[o]
        key = b_ * 2 + hi_
        cnt = np.bincount(key, minlength=NBLK * 2)
        starts = np.zeros(NBLK * 2 + 1, np.int64)
        np.cumsum(cnt, out=starts[1:])
        # per (block, half): idx values and dl columns
        for ci, (half, bs) in enumerate(calls):
            col0 = call_cols[ci]
            w = []
            for b in bs:
                kch = int((kch_lo if half == 0 else kch_hi)[b])
                g = b * 2 + half
                n = int(cnt[g])
                sl = slice(starts[g], starts[g] + n)
                iv = np.zeros(kch * P, np.int16)
                iv[:n] = (s_[sl] - (HI_BASE if half else 0)).astype(np.int16)
                w.append(iv)
                dv = np.full(kch * P, 200.0, np.float16)
                dv[:n] = d_[sl]
                cc = chunk_col[b] + (0 if half == 0 else int(kch_lo[b]))
                dl_T[c, :, cc:cc + kch] = dv.reshape(kch, P).T
            wv = np.concatenate(w)
            ww = wv.reshape(len(wv) // 16, 16).T       # wrap [i%16, i//16]
            idx_w[c, :, col0:col0 + ww.shape[1]] = np.tile(ww, (8, 1))
    meta = dict(split_s=split_s, kch_lo=kch_lo, kch_hi=kch_hi, calls=calls,
                pair_calls=pair_calls, call_cols=call_cols,
                chunk_col=chunk_col, idx_cols=idx_cols, nchunk=nchunk)
    return meta, idx_w, dl_T


def _build(meta):
    kch_lo, kch_hi = meta["kch_lo"], meta["kch_hi"]
    calls, call_cols = meta["calls"], meta["call_cols"]
    pair_calls = meta["pair_calls"]
    chunk_col = meta["chunk_col"]
    nchunk, idx_cols = meta["nchunk"], meta["idx_cols"]
    cpb = kch_lo + kch_hi
    CPBMAX = int(cpb.max())
    GMAX = max(sum(int((kch_lo if h == 0 else kch_hi)[b]) for b in bs)
               for h, bs in calls)

    nc = bacc.Bacc(None, target_bir_lowering=False)

    # ---- inputs (per core) ----
    embT_in = nc.dram_tensor("embT", [EMB, NROWS], F16, kind="ExternalInput")
    embm_in = nc.dram_tensor("embTm", [EMB, NBLK * P], F16, kind="ExternalInput")
    w1_in = nc.dram_tensor("w1ext", [EMB, 272], F16, kind="ExternalInput")
    w2_in = nc.dram_tensor("w2ext", [HIDDEN, T2_USED], F16, kind="ExternalInput")
    crow_in = nc.dram_tensor("crow", [P, T2_USED], F16, kind="ExternalInput")
    idx_in = nc.dram_tensor("idxw", [128, idx_cols], I16, kind="ExternalInput")
    dl_in = nc.dram_tensor("dlT", [128, nchunk], F16, kind="ExternalInput")
    iota_in = nc.dram_tensor("iota", [P, P], F16, kind="ExternalInput")
    out_t = nc.dram_tensor("out", [NSH, REPR], F32, kind="ExternalOutput")
    DBG = bool(os.environ.get("KDBG"))
    if DBG:
        dbg_t1 = nc.dram_tensor("dbg_t1", [256, T1_ELEM], F16, kind="ExternalOutput")
        dbg_adst1 = nc.dram_tensor("dbg_adst1", [P, NBLK * HEADS], F16, kind="ExternalOutput")
        dbg_pacc = nc.dram_tensor("dbg_pacc", [P, MC1], F32, kind="ExternalOutput")
        dbg_x16 = nc.dram_tensor("dbg_x16", [P, HIDDEN], F16, kind="ExternalOutput")
        dbg_t2s = nc.dram_tensor("dbg_t2s", [256, T2_USED], F16, kind="ExternalOutput")
        dbg_t2w = nc.dram_tensor("dbg_t2w", [256, T2_ELEM], F16, kind="ExternalOutput")

    # ---- internal DRAM ----
    t1_full = nc.dram_tensor("t1_full", [NROWS, T1_ELEM], F16, kind="Internal")
    t2_shard = nc.dram_tensor("t2_shard", [NSH, T2_USED], F16, kind="Internal")
    t2_pack = nc.dram_tensor("t2_pack", [N, T2_USED], F16, kind="Internal",
                             addr_space="Shared")
    t2_wide = nc.dram_tensor("t2_wide", [NROWS, T2_ELEM], F16, kind="Internal")

    with tile.TileContext(nc) as tc:
        with (
            tc.tile_pool(name="const", bufs=1) as cst,
            tc.tile_pool(name="sb", bufs=4) as sb,
            tc.tile_pool(name="gp", bufs=6) as gp,
            tc.tile_pool(name="rp", bufs=4) as rp,
            tc.tile_pool(name="sb2", bufs=3) as sb2,
            tc.tile_pool(name="oh", bufs=4) as ohp,
            tc.tile_pool(name="ohT", bufs=4) as ohtp,
            tc.tile_pool(name="psA", bufs=3, space="PSUM") as psA,
            tc.tile_pool(name="psB", bufs=2, space="PSUM") as psB,
            tc.tile_pool(name="psC", bufs=1, space="PSUM") as psC,
            tc.tile_pool(name="psD", bufs=2, space="PSUM") as psD,
        ):
            # ---- constants ----
            iota = cst.tile([P, P], F16)
            nc.sync.dma_start(out=iota[:], in_=iota_in[:])
            ident = cst.tile([P, P], F16)
            make_identity(nc, ident[:])
            w1 = cst.tile([EMB, 272], F16)
            nc.sync.dma_start(out=w1[:], in_=w1_in[:])
            w2 = cst.tile([P, 2, T2_USED], F16)
            nc.sync.dma_start(out=w2[:, 0, :], in_=w2_in[0:P, :])
            nc.sync.dma_start(out=w2[:, 1, :], in_=w2_in[P:HIDDEN, :])
            crow = cst.tile([P, T2_USED], F16)
            nc.sync.dma_start(out=crow[:], in_=crow_in[:])
            it_all = cst.tile([128, idx_cols], I16)
            nc.sync.dma_start(out=it_all[:], in_=idx_in[:])
            dl_all = cst.tile([128, nchunk], F16)
            nc.sync.dma_start(out=dl_all[:], in_=dl_in[:])
            adst1 = cst.tile([P, NBLK * HEADS], F16)
            adst2 = cst.tile([P, NBLK], F16)

            # ---- a_dst mini-GEMM over my own dst shard ----
            for g in range(7):
                em = sb.tile([EMB, 7, P], F16, tag="embm")
                nc.sync.dma_start(
                    out=em[:].rearrange("p a n -> p (a n)"),
                    in_=embm_in[:, g * 7 * P:(g + 1) * 7 * P])
                psx = psC.tile([P, 7, HEADS], F32, tag="pse")
                for j in range(7):
                    nc.tensor.matmul(out=psx[:, j, :], lhsT=em[:, j, :],
                                     rhs=w1[:, 264:272], start=True, stop=True)
                nc.vector.tensor_copy(
                    out=adst1[:, g * 56:(g + 1) * 56],
                    in_=psx[:].rearrange("p a h -> p (a h)"))

            # ---- phase A: replicated full-table GEMM1 (8 blocks/group) ----
            etq = {}

            def load_embT(g):
                et = sb.tile([EMB, 8, P], F16, tag="embT")
                nc.sync.dma_start(
                    out=et[:].rearrange("p a n -> p (a n)"),
                    in_=embT_in[:, g * 8 * P:(g + 1) * 8 * P])
                etq[g] = et

            load_embT(0)
            load_embT(1)
            for g in range(49):
                if g + 2 < 49:
                    load_embT(g + 2)
                et = etq.pop(g)
                t1s = sb.tile([P, 8, T1_USED], F16, tag="t1s")
                for j in range(8):
                    ph = psA.tile([P, T1_USED], F32, tag="acc")
                    nc.tensor.matmul(out=ph[:], lhsT=et[:, j, :],
                                     rhs=w1[:, 0:T1_USED], start=True, stop=True)
                    eng = (nc.vector, nc.scalar, nc.vector)[j % 3]
                    if eng is nc.scalar:
                        eng.copy(out=t1s[:, j, :], in_=ph[:])
                    else:
                        eng.tensor_copy(out=t1s[:, j, :], in_=ph[:])
                nc.sync.dma_start(
                    out=t1_full[g * 8 * P:(g + 1) * 8 * P, 0:T1_USED]
                        .rearrange("(a p) e -> p a e", p=P),
                    in_=t1s[:])

            if DBG:
                nc.sync.dma_start(out=dbg_t1[:], in_=t1_full[0:256, :])
                nc.sync.dma_start(out=dbg_adst1[:], in_=adst1[:])

            # ---- edge phase helper ----
            def edge_layer(t_full, elem, hid, heads, dmajor, adst_t, mcols,
                           out_cb):
                PREQ = 1              # pair prefetch distance
                gq = {}

                def issue_gathers(q):
                    gs = {}
                    for ci in pair_calls[q]:
                        half, bs = calls[ci]
                        kch = kch_lo if half == 0 else kch_hi
                        ch = sum(int(kch[b]) for b in bs)
                        gt = gp.tile([P, GMAX * T1_ELEM], F16, tag=f"g{half}")
                        col0 = call_cols[ci]
                        r0 = HI_BASE if half else 0
                        nc.gpsimd.dma_gather(
                            out_ap=gt[:, 0:ch * elem]
                                .rearrange("p (t e) -> p t e", e=elem),
                            in_ap=t_full[r0:NROWS, :],
                            idxs_ap=it_all[:, col0:col0 + ch * 8],
                            num_idxs=ch * P, num_idxs_reg=ch * P,
                            elem_size=elem)
                        off = 0
                        for b in bs:
                            gs[(b, half)] = (gt, off)
                            off += int(kch[b])
                    gq[q] = gs

                npairs = len(pair_calls)
                nextq = [0]

                def ensure_pairs(upto):
                    while nextq[0] <= min(upto, npairs - 1):
                        issue_gathers(nextq[0])
                        nextq[0] += 1

                def gv(b, half, c0, c1, e0, e1):
                    gt, off = gq[b // 2][(b, half)]
                    v = gt[:, 0:(off + c1) * elem].rearrange(
                        "p (t e) -> p t e", e=elem)
                    return v[:, off + c0:off + c1, e0:e1]

                state = {}

                def stage0(b):
                    """one-hots + transposed one-hots (needs only dl/iota)."""
                    t = int(cpb[b])
                    cc = int(chunk_col[b])
                    oh = ohp.tile([P, CPBMAX, P], F16, tag="oh")
                    nc.vector.tensor_tensor(
                        out=oh[:, 0:t, :],
                        in0=dl_all[:, cc:cc + t, None].to_broadcast([P, t, P]),
                        in1=iota[:, None, :].to_broadcast([P, t, P]),
                        op=ALU.is_equal)
                    ohT = ohtp.tile([P, CPBMAX, P], F16, tag="ohT")
                    for g0 in range(0, t, 8):
                        g1 = min(g0 + 8, t)
                        pst = psB.tile([P, 8, P], F16, tag="ohT")
                        for k in range(g0, g1):
                            nc.tensor.transpose(out=pst[:, k - g0, :],
                                                in_=oh[:, k, :],
                                                identity=ident[:])
                        eng = nc.scalar
                        if eng is nc.scalar:
                            eng.copy(
                                out=ohT[:, g0:g1, :].rearrange("p t n -> p (t n)"),
                                in_=pst[:, 0:g1 - g0, :]
                                    .rearrange("p t n -> p (t n)"))
                        else:
                            eng.tensor_copy(
                                out=ohT[:, g0:g1, :].rearrange("p t n -> p (t n)"),
                                in_=pst[:, 0:g1 - g0, :]
                                    .rearrange("p t n -> p (t n)"))
                    state[b] = dict(oh=oh, ohT=ohT)

                def stage1(b):
                    """logits -> exp weights (needs gathers + ohT)."""
                    st = state[b]
                    klo, khi = int(kch_lo[b]), int(kch_hi[b])
                    t = klo + khi
                    ohT = st["ohT"]
                    pse = psC.tile([P, CPBMAX * heads], F32, tag="pse")
                    for k in range(t):
                        half, ko = (0, k) if k < klo else (1, k - klo)
                        nc.tensor.matmul(
                            out=pse[:, k * heads:(k + 1) * heads],
                            lhsT=ohT[:, k, :],
                            rhs=adst_t[:, b * heads:(b + 1) * heads],
                            start=True, stop=False)
                        nc.tensor.matmul(
                            out=pse[:, k * heads:(k + 1) * heads],
                            lhsT=ident[:],
                            rhs=gv(b, half, ko, ko + 1, hid, hid + heads)[:, 0, :],
                            start=False, stop=True)
                    lk = sb2.tile([P, CPBMAX * heads], F32, tag=f"lk{hid}")
                    nc.scalar.activation(
                        out=lk[:, 0:t * heads],
                        in_=pse[:, 0:t * heads],
                        func=AF.Prelu, alpha=NEG_SLOPE)
                    rhs = rp.tile([P, CPBMAX, mcols], F16, tag="rhs")
                    nc.scalar.activation(
                        out=rhs[:, 0:t, hid:hid + heads],
                        in_=lk[:, 0:t * heads]
                            .rearrange("p (t h) -> p t h", h=heads),
                        func=AF.Exp)
                    st["rhs"] = rhs

                def stage2(b):
                    """messages + scatter + finish."""
                    st = state.pop(b)
                    klo, khi = int(kch_lo[b]), int(kch_hi[b])
                    t = klo + khi
                    oh, rhs = st["oh"], st["rhs"]
                    for half, k0, k1 in ((0, 0, klo), (1, klo, t)):
                        kk = k1 - k0
                        eng = nc.vector
                        if dmajor:
                            eng.tensor_tensor(
                                out=rhs[:, k0:k1, 0:hid]
                                    .rearrange("p t (d h) -> p t d h", h=heads),
                                in0=gv(b, half, 0, kk, 0, hid)
                                    .rearrange("p t (d h) -> p t d h", h=heads),
                                in1=rhs[:, k0:k1, None, hid:hid + heads]
                                    .to_broadcast([P, kk, hid // heads, heads]),
                                op=ALU.mult)
                        else:
                            eng.tensor_tensor(
                                out=rhs[:, k0:k1, 0:hid],
                                in0=gv(b, half, 0, kk, 0, hid),
                                in1=rhs[:, k0:k1, hid:hid + 1]
                                    .to_broadcast([P, kk, hid]),
                                op=ALU.mult)
                    pacc = psA.tile([P, mcols], F32, tag="acc")
                    for k in range(t):
                        nc.tensor.matmul(out=pacc[:], lhsT=oh[:, k, :],
                                         rhs=rhs[:, k, :], start=(k == 0),
                                         stop=(k == t - 1))
                    if b % 2 == 1:
                        gq.pop(b // 2)
                    out_cb(b, pacc)

                ensure_pairs(1)
                stage0(0)
                stage0(1)
                stage0(2)
                stage1(0)
                for b in range(NBLK):
                    ensure_pairs((b + 7) // 2)
                    if b + 3 < NBLK:
                        stage0(b + 3)
                    if b + 1 < NBLK:
                        stage1(b + 1)
                    stage2(b)

            # ---- layer 1 finisher ----
            def restride(ci):
                b0c, b1c = T2_CHUNKS[ci]
                r0, r1 = b0c * P, min(b1c * P, NSH)
                for c in range(NC):
                    nc.sync.dma_start(
                        out=t2_wide[c * NSH + r0:c * NSH + r1, 0:T2_USED],
                        in_=t2_pack[NC * r0 + c * (r1 - r0):
                                    NC * r0 + (c + 1) * (r1 - r0), :])

            t2s = {"tile": None, "b0": 0}

            def flush_t2(bend):
                tl, b0 = t2s["tile"], t2s["b0"]
                if tl is None:
                    return
                rows = min(bend * P, NSH) - b0 * P
                nc.sync.dma_start(
                    out=t2_shard[b0 * P:b0 * P + rows, :]
                        .rearrange("(a p) e -> p a e", p=P)
                    if rows % P == 0 else
                    t2_shard[b0 * P:b0 * P + rows, :],
                    in_=tl[:, 0:rows // P, :] if rows % P == 0
                    else tl[:rows, bend - 1 - b0, :])
                t2s["tile"] = None

            def finish1(b, pacc):
                if DBG and b == 0:
                    dpc = sb.tile([P, MC1], F32, tag="dpc")
                    nc.vector.tensor_copy(out=dpc[:], in_=pacc[:])
                    nc.sync.dma_start(out=dbg_pacc[:], in_=dpc[:])
                se = sb.tile([P, HEADS], F32, tag="se")
                nc.scalar.activation(out=se[:], in_=pacc[:, HIDDEN:MC1],
                                     func=AF.Copy, bias=1e-16)
                rec = sb.tile([P, HEADS], F32, tag="rec")
                nc.vector.reciprocal(out=rec[:], in_=se[:])
                v = sb.tile([P, HIDDEN], F32, tag="v")
                nc.vector.tensor_tensor(
                    out=v[:].rearrange("p (d h) -> p d h", h=HEADS),
                    in0=pacc[:, 0:HIDDEN].rearrange("p (d h) -> p d h", h=HEADS),
                    in1=rec[:, None, :].to_broadcast([P, OUT1, HEADS]),
                    op=ALU.mult)
                # elu(v) = relu(v) + exp(min(v,0)) - 1
                r = sb.tile([P, HIDDEN], F32, tag="relu")
                nc.scalar.activation(out=r[:], in_=v[:], func=AF.Relu)
                mn = sb.tile([P, HIDDEN], F32, tag="mn")
                nc.scalar.activation(out=mn[:], in_=v[:], func=AF.Relu,
                                     scale=-1.0)
                em = sb.tile([P, HIDDEN], F32, tag="em")
                nc.scalar.activation(out=em[:], in_=mn[:], func=AF.Exp,
                                     scale=-1.0)
                x = sb.tile([P, HIDDEN], F32, tag="x")
                nc.vector.tensor_tensor(out=x[:], in0=r[:], in1=em[:],
                                        op=ALU.add)
                x16 = sb.tile([P, HIDDEN], F16, tag="x16")
                nc.scalar.activation(out=x16[:], in_=x[:], func=AF.Copy,
                                     bias=-1.0)
                if DBG and b == 0:
                    nc.sync.dma_start(out=dbg_x16[:], in_=x16[:])
                xT = sb.tile([P, 2, P], F16, tag="xT")
                for k in range(2):
                    pst = psD.tile([P, P], F16, tag="misc")
                    nc.tensor.transpose(out=pst[:], in_=x16[:, k * P:(k + 1) * P],
                                        identity=ident[:])
                    nc.scalar.copy(out=xT[:, k, :], in_=pst[:])
                ph2 = psD.tile([P, T2_USED], F32, tag="misc")
                for k in range(2):
                    nc.tensor.matmul(out=ph2[:], lhsT=xT[:, k, :], rhs=w2[:, k, :],
                                     start=(k == 0), stop=(k == 1))
                if t2s["tile"] is None:
                    tl_new = sb.tile([P, 4, T2_USED], F16, tag="t2s")
                    t2s["tile"] = tl_new
                    t2s["b0"] = b
                tl = t2s["tile"]
                nc.scalar.copy(out=tl[:, b - t2s["b0"], :], in_=ph2[:])
                nc.scalar.activation(out=adst2[:, b:b + 1], in_=ph2[:, 65:66],
                                     func=AF.Copy)
                if b - t2s["b0"] == 3 or b == NBLK - 1:
                    flush_t2(b + 1)
                # t2 collective chunks ride block completion.  Collectives
                # must issue on the Pool queue; trigger each chunk 2 blocks
                # after its input flush so the Pool SEQ wait (which would
                # block later gather issues) is already satisfied.
                for ci, (b0c, b1c) in enumerate(T2_CHUNKS):
                    if b == (min(b1c + 1, NBLK - 1) if b1c <= 36 else b1c - 1):
                        r0, r1 = b0c * P, min(b1c * P, NSH)
                        nc.gpsimd.collective_compute(
                            "AllGather", ALU.bypass,
                            ins=[t2_shard[r0:r1, :]],
                            outs=[t2_pack[NC * r0:NC * r1, :]],
                            replica_groups=[list(range(NC))])
                # early chunks' restrides run mid-L1 once their collectives
                # are long done; the rest go after L1
                if b == 30:
                    restride(0)
                if b == 38:
                    restride(1)
                if b == 46:
                    restride(2)

            edge_layer(t1_full, T1_ELEM, HIDDEN, HEADS, True, adst1, MC1,
                       finish1)
            for ci in range(3, len(T2_CHUNKS)):
                restride(ci)
            if DBG:
                nc.sync.dma_start(out=dbg_t2s[:], in_=t2_shard[0:256, :])
                nc.sync.dma_start(out=dbg_t2w[:], in_=t2_wide[0:256, :])

            # ---- layer 2 finisher ----
            o4 = {"tile": None, "b0": 0}

            def finish2(b, pacc):
                se = sb.tile([P, 1], F32, tag="se2")
                nc.vector.tensor_scalar_add(out=se[:], in0=pacc[:, REPR:MC2],
                                            scalar1=1e-16)
                rec = sb.tile([P, 1], F32, tag="rec2")
                nc.vector.reciprocal(out=rec[:], in_=se[:])
                if o4["tile"] is None:
                    ot_new = sb.tile([P, 4, REPR], F32, tag="o4")
                    o4["tile"] = ot_new
                    o4["b0"] = b
                tl = o4["tile"]
                nc.scalar.activation(out=tl[:, b - o4["b0"], :],
                                     in_=pacc[:, 0:REPR], func=AF.Copy,
                                     scale=rec[:, 0:1])
                if b - o4["b0"] == 3 or b == NBLK - 1:
                    b0 = o4["b0"]
                    rows = min((b + 1) * P, NSH) - b0 * P
                    if rows % P == 0:
                        nc.sync.dma_start(
                            out=out_t[b0 * P:b0 * P + rows, :]
                                .rearrange("(a p) e -> p a e", p=P),
                            in_=tl[:, 0:rows // P, :])
                    else:
                        full = rows // P
                        if full:
                            nc.sync.dma_start(
                                out=out_t[b0 * P:b0 * P + full * P, :]
                                    .rearrange("(a p) e -> p a e", p=P),
                                in_=tl[:, 0:full, :])
                        nc.sync.dma_start(
                            out=out_t[b0 * P + full * P:b0 * P + rows, :],
                            in_=tl[:rows - full * P, full, :])
                    o4["tile"] = None

            edge_layer(t2_wide, T2_ELEM, REPR, 1, False, adst2, MC2,
                       finish2)

    nc.finalize()
    globals()["LAST_NC"] = nc
    return nc


def kernel(**inputs):
    node_emb = np.asarray(inputs["node_emb"], np.float32)
    W1 = np.asarray(inputs["W1"], np.float32)
    att1_src = np.asarray(inputs["att1_src"], np.float32)
    att1_dst = np.asarray(inputs["att1_dst"], np.float32)
    b1 = np.asarray(inputs["b1"], np.float32)
    W2 = np.asarray(inputs["W2"], np.float32)
    att2_src = np.asarray(inputs["att2_src"], np.float32)
    att2_dst = np.asarray(inputs["att2_dst"], np.float32)
    b2 = np.asarray(inputs["b2"], np.float32)
    edge_index = np.asarray(inputs["edge_index"])
    assert not np.any(b1) and not np.any(b2), "nonzero biases unsupported"

    meta, idx_w, dl_T = _prep_edges(edge_index)

    # d-major permutation: table col d*8+h <- W1 output col h*32+d
    dd, hh = np.meshgrid(np.arange(OUT1), np.arange(HEADS), indexing="ij")
    src_cols = (hh * OUT1 + dd).reshape(-1)          # length 256, j=d*8+h
    A1s = np.einsum("ehd,hd->eh", W1.reshape(EMB, HEADS, OUT1), att1_src)
    A1d = np.einsum("ehd,hd->eh", W1.reshape(EMB, HEADS, OUT1), att1_dst)
    w1ext = np.concatenate([W1[:, src_cols], A1s, A1d], axis=1).astype(np.float16)
    A2s = W2 @ att2_src[0]
    A2d = W2 @ att2_dst[0]
    w2e_rows = np.concatenate([W2, A2s[:, None], A2d[:, None]], axis=1)
    w2ext = w2e_rows[src_cols, :].astype(np.float16)   # rows follow x16 cols
    # x16 = elu(v)+1, so GEMM2 adds a constant row correction -sum(W2ext rows)
    # injected via matmul(lhsT=ident, rhs=crow).
    crow = np.tile((-w2ext.astype(np.float32).sum(axis=0))[None, :],
                   (P, 1)).astype(np.float16)

    iota = np.tile(np.arange(P, dtype=np.float16), (P, 1))

    nc = _build(meta)

    embT = np.zeros((EMB, NROWS), np.float16)
    embT[:, :N] = node_emb.T.astype(np.float16)
    embm = np.zeros((NC, EMB, NBLK * P), np.float16)
    for c in range(NC):
        embm[c, :, :NSH] = node_emb[c * NSH:(c + 1) * NSH].T.astype(np.float16)

    in_maps = []
    for c in range(NC):
        in_maps.append({
            "embT": embT,
            "embTm": embm[c],
            "w1ext": w1ext,
            "w2ext": w2ext,
            "crow": crow,
            "idxw": idx_w[c],
            "dlT": dl_T[c],
            "iota": iota,
        })

    res = run_bass_kernel_spmd(nc, in_maps, core_ids=list(range(NC)))
    globals()["LAST_RES"] = res
    globals()["LAST_META"] = meta
    out = np.concatenate([res.results[c]["out"] for c in range(NC)], axis=0)
    return np.ascontiguousarray(out.astype(np.float32))


if __name__ == "__main__":
    rng = np.random.default_rng(0)
    ins = {
        "node_emb": rng.standard_normal((N, EMB), dtype=np.float32) * 0.05,
        "W1": rng.standard_normal((EMB, HIDDEN), dtype=np.float32) * 0.07,
        "att1_src": rng.standard_normal((HEADS, OUT1), dtype=np.float32) * 0.2,
        "att1_dst": rng.standard_normal((HEADS, OUT1), dtype=np.float32) * 0.2,
        "b1": np.zeros(HIDDEN, np.float32),
        "W2": rng.standard_normal((HIDDEN, REPR), dtype=np.float32) * 0.07,
        "att2_src": rng.standard_normal((1, REPR), dtype=np.float32) * 0.2,
        "att2_dst": rng.standard_normal((1, REPR), dtype=np.float32) * 0.2,
        "b2": np.zeros(REPR, np.float32),
        "edge_index": rng.integers(0, N, (2, E)).astype(np.int32),
    }
    out = kernel(**ins)
    print("out", out.shape, out.dtype, np.abs(out).mean())
